# revision 8
# baseline (speedup 1.0000x reference)
"""Deformable Conv (DCNv2) Trainium2 Bass kernel.

Sharding: 8 cores = 2 batches x 4 H-slabs of 32 output rows each.

Per-core pipeline (single SPMD program, per-core data):
  1. offset/mask 3x3 conv as an 18-step fp32r GEMM on the PE from a
     CHW x-slab resident in SBUF.
  2. PE-transpose offsets to [pixel-partition, (row, k)] layout; compute
     bilinear blend coefficients (fp32) and gather indices (int16) with
     wide DVE/ACT ops. Validity of out-of-image corners is folded into the
     coefficients; addressing uses clamped indices, so arbitrary offsets
     are handled exactly.
  3. dma_gather (SWDGE) pulls, per (kernel-pos k, pixel), one 2KB row of a
     host-built "quad" image (4 bilinear corners x 256 channels, fp16) from
     HBM into [pixel, 4*256] SBUF tiles.
  4. DVE tensor_scalar/scalar_tensor_tensor chain blends the 4 corners with
     per-pixel (per-partition) fp32 coefficients (mask folded in) -> fp16.
  5. PE transposes blended tiles to [channel, pixel] and the main GEMM
     accumulates out[o,p] = sum_{c,k} W[o,c,k] * blended[c,k,p] in PSUM
     (fp16 x fp16 -> fp32).
"""
import os
import numpy as np
from contextlib import ExitStack

import concourse.bass as bass
import concourse.tile as tile
from concourse import bacc, mybir
from concourse.bass_utils import run_bass_kernel_spmd
from concourse.masks import make_identity
from concourse import library_config

F32 = mybir.dt.float32
F32R = mybir.dt.float32r
F16 = mybir.dt.float16
I16 = mybir.dt.int16
AF = mybir.ActivationFunctionType
OP = mybir.AluOpType

CIN = 256
COUT = 256
K2 = 9
H = W = 128
B = 2
HS = 32           # output rows per core
NCORE = 8
QD = H + 1        # quad grid dim (129)
NQ = QD * QD      # 16641 quad rows
MAGIC = 12582912.0  # 1.5 * 2**23 fp32 round-to-int magic

NBLK = 32         # pixel blocks (rows) per core
NQRT = 4          # quarters (8 rows each) per core
BPQ = 8           # blocks per quarter
# blend blocks 0..ACT_BLEND-1 use ACT for the 4 scalar muls (engine balance)
ACT_BLEND = 3


# ----------------------------------------------------------------------------
# device program
# ----------------------------------------------------------------------------

def build_program():
    nc = bacc.Bacc("TRN2", target_bir_lowering=False, debug=False,
                   num_swdge_queues=2)

    xslab = nc.dram_tensor("xslab", [2, 128, 34, 130], F32R, kind="ExternalInput")
    quad = nc.dram_tensor("quad", [NQ, 1024], F16, kind="ExternalInput")
    womt = nc.dram_tensor("womt", [18, 128, 32], F32R, kind="ExternalInput")
    wmaint = nc.dram_tensor("wmaint", [128, 36, 128], F16, kind="ExternalInput")
    baseY = nc.dram_tensor("baseY", [128, K2, NBLK], F32, kind="ExternalInput")
    baseX = nc.dram_tensor("baseX", [128, K2, NBLK], F32, kind="ExternalInput")
    bofft = nc.dram_tensor("bofft", [32, 1], F32, kind="ExternalInput")
    out = nc.dram_tensor("out", [2, 128, HS, W], F32, kind="ExternalOutput")

    with tile.TileContext(nc) as tc, ExitStack() as ctx:
        const = ctx.enter_context(tc.tile_pool(name="const", bufs=1))
        work = ctx.enter_context(tc.tile_pool(name="work", bufs=1))
        coeff = ctx.enter_context(tc.tile_pool(name="coeff", bufs=1))
        tmp = ctx.enter_context(tc.tile_pool(name="tmp", bufs=4))
        gpool = ctx.enter_context(tc.tile_pool(name="gpool", bufs=3))
        bpool = ctx.enter_context(tc.tile_pool(name="bpool", bufs=2))
        rhsp = ctx.enter_context(tc.tile_pool(name="rhsp", bufs=4))
        outp = ctx.enter_context(tc.tile_pool(name="outp", bufs=3))
        psB = ctx.enter_context(tc.tile_pool(name="psB", bufs=2, space="PSUM"))
        psC = ctx.enter_context(tc.tile_pool(name="psC", bufs=2, space="PSUM"))
        psO = ctx.enter_context(tc.tile_pool(name="psO", bufs=1, space="PSUM"))

        # ---- constants -----------------------------------------------------
        nc.gpsimd.load_library(library_config.mlp)
        ident16 = const.tile([128, 128], F16)
        make_identity(nc, ident16[:])
        identf = const.tile([128, 128], F32)
        make_identity(nc, identf[:])

        wom_sb = const.tile([128, 18, 32], F32R)
        nc.sync.dma_start(wom_sb[:], womt[:].rearrange("t c o -> c t o"))
        wm_sb = const.tile([128, 36, 128], F16)
        nc.sync.dma_start(wm_sb[:], wmaint[:])
        bY = const.tile([128, K2, NBLK], F32)
        nc.sync.dma_start(bY[:], baseY[:])
        bX = const.tile([128, K2, NBLK], F32)
        nc.sync.dma_start(bX[:], baseX[:])
        bo = const.tile([32, 1], F32)
        nc.sync.dma_start(bo[:], bofft[:])

        xs = []
        for ch in range(2):
            t = work.tile([128, 34, 130], F32R, tag=f"xs{ch}")
            nc.sync.dma_start(t[:], xslab[ch])
            xs.append(t)

        # ---- per-quarter pipeline with one-quarter lookahead ----------------
        def emit_prep(q):
            # 1. offset/mask conv for this quarter (8 rows, 2 N-blocks)
            sb_om = work.tile([32, 8 * W], F32, tag="sb_om", name="sb_om",
                              bufs=2)
            for lnb in range(2):
                nb = q * 2 + lnb
                ps = psC.tile([32, 512], F32, tag="omstage", name="ps_om")
                for t in range(18):
                    k, ch = divmod(t, 2)
                    ky, kx = divmod(k, 3)
                    rhs = xs[ch][:, nb * 4 + ky:nb * 4 + ky + 4, kx:kx + 128]
                    nc.tensor.matmul(
                        ps[:],
                        wom_sb[:, t, :],
                        rhs,
                        start=(t == 0),
                        stop=(t == 17),
                    )
                nc.scalar.activation(sb_om[:, lnb * 512:(lnb + 1) * 512],
                                     ps[:], AF.Identity, bias=bo[:])

            # 2a. transpose offsets to [pix, (blk, ch27)]
            t_off = coeff.tile([128, 27, 8], F32, tag="t_off", name="t_off",
                               bufs=2)
            for g in range(2):
                tp = psC.tile([128, 128], F32, tag="omstage", name="tp_o")
                for j in range(4):
                    bl = g * 4 + j
                    nc.tensor.transpose(
                        tp[:, j * 27:(j + 1) * 27],
                        sb_om[0:27, bl * 128:(bl + 1) * 128],
                        identf[0:27, 0:27],
                    )
                nc.scalar.copy(t_off[:, :, g * 4:(g + 1) * 4]
                               .rearrange('p c b -> p b c'), tp[:, 0:108])

            # 2b. coefficient + index pipeline (wide [128, 9, 8] ops)
            dy = t_off[:, 0:9, :]
            dx = t_off[:, 9:18, :]
            ml = t_off[:, 18:27, :]
            bYq = bY[:, :, q * 8:(q + 1) * 8]
            bXq = bX[:, :, q * 8:(q + 1) * 8]

            def ctile(tag):
                return coeff.tile([128, K2, 8], F32, tag=tag, name=tag,
                                  bufs=2)

            m = ctile('m')
            nc.scalar.activation(m[:], ml, AF.Sigmoid)

            pyp = ctile('pyp')
            nc.vector.tensor_add(pyp[:], dy, bYq)
            y0 = ctile('y0')
            nc.vector.tensor_scalar(y0[:], pyp[:], MAGIC, None, OP.add)
            nc.vector.tensor_scalar(y0[:], y0[:], -MAGIC, None, OP.add)
            wy = ctile('wy')
            nc.vector.scalar_tensor_tensor(wy[:], pyp[:], 0.5, y0[:], OP.add,
                                           OP.subtract)
            pxp = ctile('pxp')
            nc.vector.tensor_add(pxp[:], dx, bXq)
            x0 = ctile('x0')
            nc.vector.tensor_scalar(x0[:], pxp[:], MAGIC, None, OP.add)
            nc.vector.tensor_scalar(x0[:], x0[:], -MAGIC, None, OP.add)
            wx = ctile('wx')
            nc.vector.scalar_tensor_tensor(wx[:], pxp[:], 0.5, x0[:], OP.add,
                                           OP.subtract)

            def valid(srcv, lo, hi, tag):
                u = tmp.tile([128, K2, 8], F32, tag="vu", name="vu")
                nc.vector.tensor_scalar(u[:], srcv[:], lo, None, OP.is_ge)
                v = tmp.tile([128, K2, 8], F32, tag="vv", name="vv")
                nc.vector.tensor_scalar(v[:], srcv[:], hi, None, OP.is_le)
                w_ = ctile(tag)
                nc.vector.tensor_mul(w_[:], u[:], v[:])
                return w_

            vy0 = valid(y0, 0.0, float(H - 1), 'vy0')
            vy1 = valid(y0, -1.0, float(H - 2), 'vy1')
            vx0 = valid(x0, 0.0, float(W - 1), 'vx0')
            vx1 = valid(x0, -1.0, float(W - 2), 'vx1')

            y0c = ctile('y0c')
            nc.vector.tensor_scalar(y0c[:], y0[:], -1.0, float(H - 1),
                                    OP.max, OP.min)
            x0c = ctile('x0c')
            nc.vector.tensor_scalar(x0c[:], x0[:], -1.0, float(W - 1),
                                    OP.max, OP.min)

            def lerp_w(wv, v0, v1, tag):
                t_ = tmp.tile([128, K2, 8], F32, tag="lw", name="lw")
                nc.vector.tensor_mul(t_[:], wv[:], v0[:])
                f0 = ctile(tag + '0')
                nc.vector.tensor_sub(f0[:], v0[:], t_[:])
                f1 = ctile(tag + '1')
                nc.vector.tensor_mul(f1[:], wv[:], v1[:])
                return f0, f1

            fy0, fy1 = lerp_w(wy, vy0, vy1, 'fy')
            fx0, fx1 = lerp_w(wx, vx0, vx1, 'fx')

            g0 = ctile('g0')
            nc.vector.tensor_mul(g0[:], fy0[:], m[:])
            g1 = ctile('g1')
            nc.vector.tensor_mul(g1[:], fy1[:], m[:])
            a00 = ctile('a00')
            nc.vector.tensor_mul(a00[:], g0[:], fx0[:])
            a01 = ctile('a01')
            nc.vector.tensor_mul(a01[:], g0[:], fx1[:])
            a10 = ctile('a10')
            nc.vector.tensor_mul(a10[:], g1[:], fx0[:])
            a11 = ctile('a11')
            nc.vector.tensor_mul(a11[:], g1[:], fx1[:])

            idxf = ctile('idxf')
            nc.vector.scalar_tensor_tensor(idxf[:], y0c[:], float(QD), x0c[:],
                                           OP.mult, OP.add)
            nc.vector.tensor_scalar(idxf[:], idxf[:], float(QD + 1), None,
                                    OP.add)
            idx16 = coeff.tile([128, K2, 8], I16, tag="idx16", name="idx16",
                               bufs=2)
            nc.vector.tensor_copy(idx16[:], idxf[:])

            # 2c. fold idx to gather layout [16, (k, blk, g)] + replicate.
            # The gather ucode (queue 0) reads idxs from partitions 0-31
            # only (cores 0-1), so replicate just that far.
            idxg = coeff.tile([128, K2, 8, 8], I16, tag="idxg", name="idxg",
                              bufs=2)
            for g in range(8):
                srcv = idx16[g * 16:(g + 1) * 16, :, :]
                nc.sync.dma_start(idxg[0:16, :, :, g], srcv)
            nc.sync.dma_start(idxg[16:32], idxg[0:16])
            nc.sync.dma_start(idxg[32:64], idxg[0:32])

            # fp16 copies of the blend coefficients (broadcast-TT operands)
            af = []
            for j, a in enumerate((a00, a01, a10)):
                t = coeff.tile([128, K2, 8], F16, tag=f"af{j}",
                               name=f"af{j}", bufs=2)
                nc.scalar.copy(t[:], a[:])
                af.append(t)

            return (a00, a01, a10, a11), af, idxg

        def emit_main(q, coefs):
            afull, af, idxg = coefs
            # 3-5. gather / blend / transpose+sum (PE) / GEMM
            po = [psO.tile([128, 512], F32, tag=f"po{i}", name=f"po{i}")
                  for i in range(4)]
            for k in range(K2):
                gbuf = gpool.tile([128, BPQ, 1024], F16, tag="gbuf")
                nc.gpsimd.dma_gather(
                    gbuf[:],
                    quad[:],
                    idxg[:, k, :, :],
                    num_idxs=BPQ * 128,
                    num_idxs_reg=BPQ * 128,
                    elem_size=1024,
                    single_packet=False,
                    queue_num=k % 2,
                )
                # scaled corners tj[j] = gbuf[:, :, j] * a_j.  Corner 3 runs
                # as narrow per-block scale-copies on ACT; corners 0-2 as
                # wide single-port broadcast-TT muls on DVE (the 2-port TS
                # path would block on the SWDGE shared-port lock during
                # gathers).  The 4-corner sum happens on the PE: identity
                # matmuls accumulate the transposed corners in PSUM.
                tj = [bpool.tile([128, BPQ, 256], F16, tag=f"tj{j}",
                                 name=f"tj{j}", bufs=3) for j in range(4)]
                for bl in range(BPQ):
                    nc.scalar.activation(tj[3][:, bl, :],
                                         gbuf[:, bl, 768:1024], AF.Copy,
                                         scale=afull[3][:, k, bl:bl + 1])
                for j in range(3):
                    nc.vector.tensor_tensor(
                        tj[j][:], gbuf[:, :, j * 256:(j + 1) * 256],
                        af[j][:, k, :, None].broadcast_to([128, BPQ, 256]),
                        OP.mult)

                for j2 in range(2):
                    for ct in range(2):
                        tp = psB.tile([128, 512], F32, tag="stage",
                                      name="tp_b")
                        for r in range(4):
                            bl = j2 * 4 + r
                            for j in range(4):
                                nc.tensor.matmul(
                                    tp[:, r * 128:(r + 1) * 128],
                                    tj[j][:, bl, ct * 128:(ct + 1) * 128],
                                    ident16[:],
                                    start=(j == 0),
                                    stop=(j == 3),
                                )
                        rhs16 = rhsp.tile([128, 512], F16, tag="rhs",
                                          name="rhs")
                        nc.scalar.copy(rhs16[:], tp[:])
                        for ot in range(2):
                            widx = (k * 2 + ct) * 2 + ot
                            nc.tensor.matmul(
                                po[j2 * 2 + ot][:],
                                wm_sb[:, widx, :],
                                rhs16[:],
                                start=(k == 0 and ct == 0),
                                stop=(k == 8 and ct == 1),
                            )
            for j2 in range(2):
                og = q * 2 + j2
                for ot in range(2):
                    o_sb = outp.tile([128, 4, 128], F32, tag="osb")
                    nc.scalar.copy(o_sb[:], po[j2 * 2 + ot][:])
                    nc.sync.dma_start(out[ot, :, og * 4:(og + 1) * 4, :], o_sb[:])

        prev = None
        for q in range(NQRT):
            coefs = emit_prep(q)
            if prev is not None:
                emit_main(prev[0], prev[1])
            prev = (q, coefs)
        emit_main(prev[0], prev[1])

    nc.finalize()
    return nc


# ----------------------------------------------------------------------------
# host-side data prep
# ----------------------------------------------------------------------------

def build_in_maps(x, w_conv, b_conv, w_off, b_off, w_mask, b_mask):
    x = np.ascontiguousarray(x, np.float32)

    # quad image per batch: quad[yq*129+xq, (j,c)] fp16
    quads = []
    for b in range(B):
        xp = np.zeros((H + 2, W + 2, CIN), np.float32)
        xp[1:-1, 1:-1] = x[b].transpose(1, 2, 0)
        q = np.empty((QD, QD, 4, CIN), np.float16)
        q[:, :, 0] = xp[0:QD, 0:QD]
        q[:, :, 1] = xp[0:QD, 1:QD + 1]
        q[:, :, 2] = xp[1:QD + 1, 0:QD]
        q[:, :, 3] = xp[1:QD + 1, 1:QD + 1]
        quads.append(np.ascontiguousarray(q.reshape(NQ, 1024)))

    # offset/mask weights, output channels reordered to [dy*9, dx*9, ml*9]
    wom = np.concatenate([w_off, w_mask], 0).reshape(27, CIN, K2)  # [o,c,k]
    perm = np.concatenate([np.arange(0, 18, 2), np.arange(1, 18, 2),
                           np.arange(18, 27)])
    womp = wom[perm]                                   # [27(dy,dx,ml), c, k]
    womt = np.zeros((18, 128, 32), np.float32)
    for t in range(18):
        k, ch = divmod(t, 2)
        womt[t, :, 0:27] = womp[:, ch * 128:(ch + 1) * 128, k].T
    bom = np.concatenate([b_off, b_mask]).astype(np.float32)[perm]
    bofft = np.zeros((32, 1), np.float32)
    bofft[0:27, 0] = bom

    # main weights [c, (k,ct,ot), o] fp16
    wc = w_conv.reshape(COUT, CIN, K2)
    wmaint = np.zeros((128, 36, 128), np.float16)
    for k in range(K2):
        for ct in range(2):
            for ot in range(2):
                widx = (k * 2 + ct) * 2 + ot
                wmaint[:, widx, :] = (
                    wc[ot * 128:(ot + 1) * 128, ct * 128:(ct + 1) * 128, k].T
                )

    ky = (np.arange(K2) // 3).astype(np.float32)
    kx = (np.arange(K2) % 3).astype(np.float32)
    bXc = np.zeros((128, K2, NBLK), np.float32)
    bXc[:] = (np.arange(128, dtype=np.float32)[:, None, None]
              + kx[None, :, None] - 1.5)

    in_maps = []
    for core in range(NCORE):
        b, slab = divmod(core, 4)
        h0 = slab * HS
        xsl = np.zeros((2, 128, 34, 130), np.float32)
        r_lo = max(0, h0 - 1)
        r_hi = min(H, h0 + HS + 1)
        xsl[:, :, (r_lo - (h0 - 1)):(r_hi - (h0 - 1)), 1:129] = (
            x[b].reshape(2, 128, H, W)[:, :, r_lo:r_hi, :]
        )
        bYc = np.zeros((128, K2, NBLK), np.float32)
        bYc[:] = ((h0 + np.arange(NBLK, dtype=np.float32))[None, None, :]
                  + ky[None, :, None] - 1.5)
        in_maps.append({
            "xslab": xsl,
            "quad": quads[b],
            "womt": womt,
            "wmaint": wmaint,
            "baseY": bYc,
            "baseX": np.ascontiguousarray(bXc),
            "bofft": bofft,
        })
    return in_maps


_PROGRAM = None
LAST_EXEC_NS = None
LAST_RESULTS = None


def kernel(x, w_conv, b_conv, w_off, b_off, w_mask, b_mask):
    global _PROGRAM, LAST_EXEC_NS, LAST_RESULTS
    in_maps = build_in_maps(x, w_conv, b_conv, w_off, b_off, w_mask, b_mask)
    if _PROGRAM is None:
        _PROGRAM = build_program()
    nc = _PROGRAM
    trace = bool(os.environ.get("DCN_TRACE"))
    res = run_bass_kernel_spmd(nc, in_maps, core_ids=list(range(NCORE)),
                               trace=trace)
    LAST_EXEC_NS = res.exec_time_ns
    LAST_RESULTS = res
    out = np.zeros((B, COUT, H, W), np.float32)
    for core in range(NCORE):
        b, slab = divmod(core, 4)
        h0 = slab * HS
        oc = res.results[core]["out"]  # [2, 128, 32, 128]
        out[b, 0:128, h0:h0 + HS, :] = oc[0]
        out[b, 128:256, h0:h0 + HS, :] = oc[1]
    # b_conv is zeros in the reference setup, but add anyway for correctness
    out += np.asarray(b_conv, np.float32)[None, :, None, None]
    return out



# revision 10
# speedup vs baseline: 1.1637x; 1.1637x over previous
"""Deformable Conv (DCNv2) Trainium2 Bass kernel.

Sharding: 8 cores = 2 batches x 4 H-slabs of 32 output rows each.

Per-core pipeline (single SPMD program, per-core data):
  1. offset/mask 3x3 conv as an 18-step fp32r GEMM on the PE from a
     CHW x-slab resident in SBUF.
  2. PE-transpose offsets to [pixel-partition, (row, k)] layout; compute
     bilinear blend coefficients (fp32) and gather indices (int16) with
     wide DVE/ACT ops. Validity of out-of-image corners is folded into the
     coefficients; addressing uses clamped indices, so arbitrary offsets
     are handled exactly.
  3. dma_gather (SWDGE) pulls, per (kernel-pos k, pixel), one 2KB row of a
     host-built "quad" image (4 bilinear corners x 256 channels, fp16) from
     HBM into [pixel, 4*256] SBUF tiles.
  4. DVE tensor_scalar/scalar_tensor_tensor chain blends the 4 corners with
     per-pixel (per-partition) fp32 coefficients (mask folded in) -> fp16.
  5. PE transposes blended tiles to [channel, pixel] and the main GEMM
     accumulates out[o,p] = sum_{c,k} W[o,c,k] * blended[c,k,p] in PSUM
     (fp16 x fp16 -> fp32).
"""
import os
import numpy as np
from contextlib import ExitStack

import concourse.bass as bass
import concourse.tile as tile
from concourse import bacc, mybir
from concourse.bass_utils import run_bass_kernel_spmd
from concourse.masks import make_identity
from concourse import library_config

F32 = mybir.dt.float32
F32R = mybir.dt.float32r
F16 = mybir.dt.float16
I16 = mybir.dt.int16
AF = mybir.ActivationFunctionType
OP = mybir.AluOpType

CIN = 256
COUT = 256
K2 = 9
H = W = 128
B = 2
HS = 32           # output rows per core
NCORE = 8
QD = H + 1        # quad grid dim (129)
NQ = QD * QD      # 16641 quad rows
MAGIC = 12582912.0  # 1.5 * 2**23 fp32 round-to-int magic

NBLK = 32         # pixel blocks (rows) per core
NQRT = 4          # quarters (8 rows each) per core
BPQ = 8           # blocks per quarter
# blend blocks 0..ACT_BLEND-1 use ACT for the 4 scalar muls (engine balance)
ACT_BLEND = 3


# ----------------------------------------------------------------------------
# device program
# ----------------------------------------------------------------------------

def build_program():
    nc = bacc.Bacc("TRN2", target_bir_lowering=False, debug=False)

    xslab = nc.dram_tensor("xslab", [2, 128, 34, 130], F32R, kind="ExternalInput")
    quad = nc.dram_tensor("quad", [NQ, 1024], F16, kind="ExternalInput")
    womt = nc.dram_tensor("womt", [18, 128, 32], F32R, kind="ExternalInput")
    wmaint = nc.dram_tensor("wmaint", [128, 36, 128], F16, kind="ExternalInput")
    baseY = nc.dram_tensor("baseY", [128, K2, NBLK], F32, kind="ExternalInput")
    baseX = nc.dram_tensor("baseX", [128, K2, NBLK], F32, kind="ExternalInput")
    bofft = nc.dram_tensor("bofft", [32, 1], F32, kind="ExternalInput")
    out = nc.dram_tensor("out", [2, 128, HS, W], F32, kind="ExternalOutput")

    with tile.TileContext(nc) as tc, ExitStack() as ctx:
        const = ctx.enter_context(tc.tile_pool(name="const", bufs=1))
        work = ctx.enter_context(tc.tile_pool(name="work", bufs=1))
        coeff = ctx.enter_context(tc.tile_pool(name="coeff", bufs=1))
        tmp = ctx.enter_context(tc.tile_pool(name="tmp", bufs=4))
        gpool = ctx.enter_context(tc.tile_pool(name="gpool", bufs=3))
        bpool = ctx.enter_context(tc.tile_pool(name="bpool", bufs=2))
        rhsp = ctx.enter_context(tc.tile_pool(name="rhsp", bufs=3))
        outp = ctx.enter_context(tc.tile_pool(name="outp", bufs=3))
        psB = ctx.enter_context(tc.tile_pool(name="psB", bufs=2, space="PSUM"))
        psC = ctx.enter_context(tc.tile_pool(name="psC", bufs=2, space="PSUM"))
        psO = ctx.enter_context(tc.tile_pool(name="psO", bufs=1, space="PSUM"))

        # ---- constants -----------------------------------------------------
        nc.gpsimd.load_library(library_config.mlp)
        ident16 = const.tile([128, 128], F16)
        make_identity(nc, ident16[:])
        identf = const.tile([128, 128], F32)
        make_identity(nc, identf[:])

        wom_sb = const.tile([128, 18, 32], F32R)
        nc.sync.dma_start(wom_sb[:], womt[:].rearrange("t c o -> c t o"))
        wm_sb = const.tile([128, 36, 128], F16)
        nc.sync.dma_start(wm_sb[:], wmaint[:])
        bY = const.tile([128, K2, NBLK], F32)
        nc.sync.dma_start(bY[:], baseY[:])
        bX = const.tile([128, K2, NBLK], F32)
        nc.sync.dma_start(bX[:], baseX[:])
        bo = const.tile([32, 1], F32)
        nc.sync.dma_start(bo[:], bofft[:])

        # broadcast-constant columns: [MAGIC, -MAGIC, 0, -1, 127, 126, 130]
        cst = const.tile([128, 8], F32)
        for i, v in enumerate((MAGIC, -MAGIC, 0.0, -1.0, float(H - 1),
                               float(H - 2), float(QD + 1))):
            nc.vector.memset(cst[:, i:i + 1], v)

        def cb(i):
            return cst[:, i:i + 1, None].broadcast_to([128, K2, 8])

        xs = []
        for ch in range(2):
            t = work.tile([128, 34, 130], F32R, tag=f"xs{ch}")
            nc.sync.dma_start(t[:], xslab[ch])
            xs.append(t)

        # ---- per-quarter pipeline with one-quarter lookahead ----------------
        def emit_prep(q):
            # 1. offset/mask conv for this quarter (8 rows, 2 N-blocks)
            sb_om = work.tile([32, 8 * W], F32, tag="sb_om", name="sb_om",
                              bufs=2)
            for lnb in range(2):
                nb = q * 2 + lnb
                ps = psC.tile([32, 512], F32, tag="omstage", name="ps_om")
                for t in range(18):
                    k, ch = divmod(t, 2)
                    ky, kx = divmod(k, 3)
                    rhs = xs[ch][:, nb * 4 + ky:nb * 4 + ky + 4, kx:kx + 128]
                    nc.tensor.matmul(
                        ps[:],
                        wom_sb[:, t, :],
                        rhs,
                        start=(t == 0),
                        stop=(t == 17),
                    )
                nc.scalar.activation(sb_om[:, lnb * 512:(lnb + 1) * 512],
                                     ps[:], AF.Identity, bias=bo[:])

            # 2a. transpose offsets to [pix, (blk, ch27)]
            t_off = coeff.tile([128, 27, 8], F32, tag="t_off", name="t_off",
                               bufs=2)
            for g in range(2):
                tp = psC.tile([128, 128], F32, tag="omstage", name="tp_o")
                for j in range(4):
                    bl = g * 4 + j
                    nc.tensor.transpose(
                        tp[:, j * 27:(j + 1) * 27],
                        sb_om[0:27, bl * 128:(bl + 1) * 128],
                        identf[0:27, 0:27],
                    )
                nc.scalar.copy(t_off[:, :, g * 4:(g + 1) * 4]
                               .rearrange('p c b -> p b c'), tp[:, 0:108])

            # 2b. coefficient + index pipeline (wide [128, 9, 8] ops)
            dy = t_off[:, 0:9, :]
            dx = t_off[:, 9:18, :]
            ml = t_off[:, 18:27, :]
            bYq = bY[:, :, q * 8:(q + 1) * 8]
            bXq = bX[:, :, q * 8:(q + 1) * 8]

            def ctile(tag):
                return coeff.tile([128, K2, 8], F32, tag=tag, name=tag,
                                  bufs=2)

            m = ctile('m')
            nc.scalar.activation(m[:], ml, AF.Sigmoid)

            pyp = ctile('pyp')
            nc.vector.tensor_add(pyp[:], dy, bYq)
            y0 = ctile('y0')
            nc.vector.tensor_tensor(y0[:], pyp[:], cb(0), OP.add)
            nc.vector.tensor_tensor(y0[:], y0[:], cb(1), OP.add)
            wy = ctile('wy')
            nc.vector.scalar_tensor_tensor(wy[:], pyp[:], 0.5, y0[:], OP.add,
                                           OP.subtract)
            pxp = ctile('pxp')
            nc.vector.tensor_add(pxp[:], dx, bXq)
            x0 = ctile('x0')
            nc.vector.tensor_tensor(x0[:], pxp[:], cb(0), OP.add)
            nc.vector.tensor_tensor(x0[:], x0[:], cb(1), OP.add)
            wx = ctile('wx')
            nc.vector.scalar_tensor_tensor(wx[:], pxp[:], 0.5, x0[:], OP.add,
                                           OP.subtract)

            def valid(srcv, lo_i, hi_i, tag):
                u = tmp.tile([128, K2, 8], F32, tag="vu", name="vu")
                nc.vector.tensor_tensor(u[:], srcv[:], cb(lo_i), OP.is_ge)
                v = tmp.tile([128, K2, 8], F32, tag="vv", name="vv")
                nc.vector.tensor_tensor(v[:], srcv[:], cb(hi_i), OP.is_le)
                w_ = ctile(tag)
                nc.vector.tensor_mul(w_[:], u[:], v[:])
                return w_

            vy0 = valid(y0, 2, 4, 'vy0')
            vy1 = valid(y0, 3, 5, 'vy1')
            vx0 = valid(x0, 2, 4, 'vx0')
            vx1 = valid(x0, 3, 5, 'vx1')

            y0c = ctile('y0c')
            nc.vector.tensor_tensor(y0c[:], y0[:], cb(3), OP.max)
            nc.vector.tensor_tensor(y0c[:], y0c[:], cb(4), OP.min)
            x0c = ctile('x0c')
            nc.vector.tensor_tensor(x0c[:], x0[:], cb(3), OP.max)
            nc.vector.tensor_tensor(x0c[:], x0c[:], cb(4), OP.min)

            def lerp_w(wv, v0, v1, tag):
                t_ = tmp.tile([128, K2, 8], F32, tag="lw", name="lw")
                nc.vector.tensor_mul(t_[:], wv[:], v0[:])
                f0 = ctile(tag + '0')
                nc.vector.tensor_sub(f0[:], v0[:], t_[:])
                f1 = ctile(tag + '1')
                nc.vector.tensor_mul(f1[:], wv[:], v1[:])
                return f0, f1

            fy0, fy1 = lerp_w(wy, vy0, vy1, 'fy')
            fx0, fx1 = lerp_w(wx, vx0, vx1, 'fx')

            g0 = ctile('g0')
            nc.vector.tensor_mul(g0[:], fy0[:], m[:])
            g1 = ctile('g1')
            nc.vector.tensor_mul(g1[:], fy1[:], m[:])
            a00 = ctile('a00')
            nc.vector.tensor_mul(a00[:], g0[:], fx0[:])
            a01 = ctile('a01')
            nc.vector.tensor_mul(a01[:], g0[:], fx1[:])
            a10 = ctile('a10')
            nc.vector.tensor_mul(a10[:], g1[:], fx0[:])
            a11 = ctile('a11')
            nc.vector.tensor_mul(a11[:], g1[:], fx1[:])

            idxf = ctile('idxf')
            nc.vector.scalar_tensor_tensor(idxf[:], y0c[:], float(QD), x0c[:],
                                           OP.mult, OP.add)
            nc.vector.tensor_tensor(idxf[:], idxf[:], cb(6), OP.add)
            idx16 = coeff.tile([128, K2, 8], I16, tag="idx16", name="idx16",
                               bufs=2)
            nc.vector.tensor_tensor(idx16[:], idxf[:], cb(2), OP.add)

            # 2c. fold idx to gather layout [16, (k, blk, g)] + replicate.
            # The gather ucode (queue 0) reads idxs from partitions 0-31
            # only (cores 0-1), so replicate just that far.
            idxg = coeff.tile([128, K2, 8, 8], I16, tag="idxg", name="idxg",
                              bufs=2)
            for g in range(8):
                srcv = idx16[g * 16:(g + 1) * 16, :, :]
                nc.sync.dma_start(idxg[0:16, :, :, g], srcv)
            nc.sync.dma_start(idxg[16:32], idxg[0:16])

            # fp16 copies of the blend coefficients (broadcast-TT operands)
            af = []
            for j, a in enumerate((a00, a01, a10)):
                t = coeff.tile([128, K2, 8], F16, tag=f"af{j}",
                               name=f"af{j}", bufs=2)
                nc.scalar.copy(t[:], a[:])
                af.append(t)

            return (a00, a01, a10, a11), af, idxg

        def emit_main(q, coefs):
            afull, af, idxg = coefs
            # 3-5. gather / blend / transpose+sum (PE) / GEMM
            po = [psO.tile([128, 512], F32, tag=f"po{i}", name=f"po{i}")
                  for i in range(4)]
            for k in range(K2):
                gbuf = gpool.tile([128, BPQ, 1024], F16, tag="gbuf")
                nc.gpsimd.dma_gather(
                    gbuf[:],
                    quad[:],
                    idxg[:, k, :, :],
                    num_idxs=BPQ * 128,
                    num_idxs_reg=BPQ * 128,
                    elem_size=1024,
                    single_packet=False,
                )
                # scaled corners tj[j] = gbuf[:, :, j] * a_j.  Corner 3 runs
                # as narrow per-block scale-copies on ACT; corners 0-2 as
                # wide single-port broadcast-TT muls on DVE (the 2-port TS
                # path would block on the SWDGE shared-port lock during
                # gathers).  The 4-corner sum happens on the PE: identity
                # matmuls accumulate the transposed corners in PSUM.
                tj = [bpool.tile([128, BPQ, 256], F16, tag=f"tj{j}",
                                 name=f"tj{j}", bufs=2) for j in range(4)]
                for bl in range(BPQ):
                    nc.scalar.activation(tj[3][:, bl, :],
                                         gbuf[:, bl, 768:1024], AF.Copy,
                                         scale=afull[3][:, k, bl:bl + 1])
                for j in range(3):
                    nc.vector.tensor_tensor(
                        tj[j][:], gbuf[:, :, j * 256:(j + 1) * 256],
                        af[j][:, k, :, None].broadcast_to([128, BPQ, 256]),
                        OP.mult)

                for j2 in range(2):
                    for ct in range(2):
                        tp = psB.tile([128, 512], F32, tag="stage",
                                      name="tp_b")
                        for r in range(4):
                            bl = j2 * 4 + r
                            for j in range(4):
                                nc.tensor.matmul(
                                    tp[:, r * 128:(r + 1) * 128],
                                    tj[j][:, bl, ct * 128:(ct + 1) * 128],
                                    ident16[:],
                                    start=(j == 0),
                                    stop=(j == 3),
                                )
                        rhs16 = rhsp.tile([128, 512], F16, tag="rhs",
                                          name="rhs")
                        nc.scalar.copy(rhs16[:], tp[:])
                        for ot in range(2):
                            widx = (k * 2 + ct) * 2 + ot
                            nc.tensor.matmul(
                                po[j2 * 2 + ot][:],
                                wm_sb[:, widx, :],
                                rhs16[:],
                                start=(k == 0 and ct == 0),
                                stop=(k == 8 and ct == 1),
                            )
            for j2 in range(2):
                og = q * 2 + j2
                for ot in range(2):
                    o_sb = outp.tile([128, 4, 128], F32, tag="osb")
                    nc.scalar.copy(o_sb[:], po[j2 * 2 + ot][:])
                    nc.sync.dma_start(out[ot, :, og * 4:(og + 1) * 4, :], o_sb[:])

        prev = None
        for q in range(NQRT):
            coefs = emit_prep(q)
            if prev is not None:
                emit_main(prev[0], prev[1])
            prev = (q, coefs)
        emit_main(prev[0], prev[1])

    nc.finalize()
    return nc


# ----------------------------------------------------------------------------
# host-side data prep
# ----------------------------------------------------------------------------

def build_in_maps(x, w_conv, b_conv, w_off, b_off, w_mask, b_mask):
    x = np.ascontiguousarray(x, np.float32)

    # quad image per batch: quad[yq*129+xq, (j,c)] fp16
    quads = []
    for b in range(B):
        xp = np.zeros((H + 2, W + 2, CIN), np.float32)
        xp[1:-1, 1:-1] = x[b].transpose(1, 2, 0)
        q = np.empty((QD, QD, 4, CIN), np.float16)
        q[:, :, 0] = xp[0:QD, 0:QD]
        q[:, :, 1] = xp[0:QD, 1:QD + 1]
        q[:, :, 2] = xp[1:QD + 1, 0:QD]
        q[:, :, 3] = xp[1:QD + 1, 1:QD + 1]
        quads.append(np.ascontiguousarray(q.reshape(NQ, 1024)))

    # offset/mask weights, output channels reordered to [dy*9, dx*9, ml*9]
    wom = np.concatenate([w_off, w_mask], 0).reshape(27, CIN, K2)  # [o,c,k]
    perm = np.concatenate([np.arange(0, 18, 2), np.arange(1, 18, 2),
                           np.arange(18, 27)])
    womp = wom[perm]                                   # [27(dy,dx,ml), c, k]
    womt = np.zeros((18, 128, 32), np.float32)
    for t in range(18):
        k, ch = divmod(t, 2)
        womt[t, :, 0:27] = womp[:, ch * 128:(ch + 1) * 128, k].T
    bom = np.concatenate([b_off, b_mask]).astype(np.float32)[perm]
    bofft = np.zeros((32, 1), np.float32)
    bofft[0:27, 0] = bom

    # main weights [c, (k,ct,ot), o] fp16
    wc = w_conv.reshape(COUT, CIN, K2)
    wmaint = np.zeros((128, 36, 128), np.float16)
    for k in range(K2):
        for ct in range(2):
            for ot in range(2):
                widx = (k * 2 + ct) * 2 + ot
                wmaint[:, widx, :] = (
                    wc[ot * 128:(ot + 1) * 128, ct * 128:(ct + 1) * 128, k].T
                )

    ky = (np.arange(K2) // 3).astype(np.float32)
    kx = (np.arange(K2) % 3).astype(np.float32)
    bXc = np.zeros((128, K2, NBLK), np.float32)
    bXc[:] = (np.arange(128, dtype=np.float32)[:, None, None]
              + kx[None, :, None] - 1.5)

    in_maps = []
    for core in range(NCORE):
        b, slab = divmod(core, 4)
        h0 = slab * HS
        xsl = np.zeros((2, 128, 34, 130), np.float32)
        r_lo = max(0, h0 - 1)
        r_hi = min(H, h0 + HS + 1)
        xsl[:, :, (r_lo - (h0 - 1)):(r_hi - (h0 - 1)), 1:129] = (
            x[b].reshape(2, 128, H, W)[:, :, r_lo:r_hi, :]
        )
        bYc = np.zeros((128, K2, NBLK), np.float32)
        bYc[:] = ((h0 + np.arange(NBLK, dtype=np.float32))[None, None, :]
                  + ky[None, :, None] - 1.5)
        in_maps.append({
            "xslab": xsl,
            "quad": quads[b],
            "womt": womt,
            "wmaint": wmaint,
            "baseY": bYc,
            "baseX": np.ascontiguousarray(bXc),
            "bofft": bofft,
        })
    return in_maps


_PROGRAM = None
LAST_EXEC_NS = None
LAST_RESULTS = None


def kernel(x, w_conv, b_conv, w_off, b_off, w_mask, b_mask):
    global _PROGRAM, LAST_EXEC_NS, LAST_RESULTS
    in_maps = build_in_maps(x, w_conv, b_conv, w_off, b_off, w_mask, b_mask)
    if _PROGRAM is None:
        _PROGRAM = build_program()
    nc = _PROGRAM
    trace = bool(os.environ.get("DCN_TRACE"))
    res = run_bass_kernel_spmd(nc, in_maps, core_ids=list(range(NCORE)),
                               trace=trace)
    LAST_EXEC_NS = res.exec_time_ns
    LAST_RESULTS = res
    out = np.zeros((B, COUT, H, W), np.float32)
    for core in range(NCORE):
        b, slab = divmod(core, 4)
        h0 = slab * HS
        oc = res.results[core]["out"]  # [2, 128, 32, 128]
        out[b, 0:128, h0:h0 + HS, :] = oc[0]
        out[b, 128:256, h0:h0 + HS, :] = oc[1]
    # b_conv is zeros in the reference setup, but add anyway for correctness
    out += np.asarray(b_conv, np.float32)[None, :, None, None]
    return out



# revision 11
# speedup vs baseline: 1.2208x; 1.0491x over previous
"""Deformable Conv (DCNv2) Trainium2 Bass kernel.

Sharding: 8 cores = 2 batches x 4 H-slabs of 32 output rows each.

Per-core pipeline (single SPMD program, per-core data):
  1. offset/mask 3x3 conv as an 18-step fp32r GEMM on the PE from a
     CHW x-slab resident in SBUF.
  2. PE-transpose offsets to [pixel-partition, (row, k)] layout; compute
     bilinear blend coefficients (fp32) and gather indices (int16) with
     wide DVE/ACT ops. Validity of out-of-image corners is folded into the
     coefficients; addressing uses clamped indices, so arbitrary offsets
     are handled exactly.
  3. dma_gather (SWDGE) pulls, per (kernel-pos k, pixel), one 2KB row of a
     host-built "quad" image (4 bilinear corners x 256 channels, fp16) from
     HBM into [pixel, 4*256] SBUF tiles.
  4. DVE tensor_scalar/scalar_tensor_tensor chain blends the 4 corners with
     per-pixel (per-partition) fp32 coefficients (mask folded in) -> fp16.
  5. PE transposes blended tiles to [channel, pixel] and the main GEMM
     accumulates out[o,p] = sum_{c,k} W[o,c,k] * blended[c,k,p] in PSUM
     (fp16 x fp16 -> fp32).
"""
import os
import numpy as np
from contextlib import ExitStack

import concourse.bass as bass
import concourse.tile as tile
from concourse import bacc, mybir
from concourse.bass_utils import run_bass_kernel_spmd
from concourse.masks import make_identity
from concourse import library_config

F32 = mybir.dt.float32
F32R = mybir.dt.float32r
F16 = mybir.dt.float16
I16 = mybir.dt.int16
AF = mybir.ActivationFunctionType
OP = mybir.AluOpType

CIN = 256
COUT = 256
K2 = 9
H = W = 128
B = 2
HS = 32           # output rows per core
NCORE = 8
QD = H + 1        # quad grid dim (129)
NQ = QD * QD      # 16641 quad rows
MAGIC = 12582912.0  # 1.5 * 2**23 fp32 round-to-int magic

NBLK = 32         # pixel blocks (rows) per core
NQRT = 4          # quarters (8 rows each) per core
BPQ = 8           # blocks per quarter
# blend blocks 0..ACT_BLEND-1 use ACT for the 4 scalar muls (engine balance)
ACT_BLEND = 3


# ----------------------------------------------------------------------------
# device program
# ----------------------------------------------------------------------------

def build_program():
    nc = bacc.Bacc("TRN2", target_bir_lowering=False, debug=False)

    xslab = nc.dram_tensor("xslab", [2, 128, 34, 130], F32R, kind="ExternalInput")
    quad = nc.dram_tensor("quad", [NQ, 1024], F16, kind="ExternalInput")
    womt = nc.dram_tensor("womt", [18, 128, 32], F32R, kind="ExternalInput")
    wmaint = nc.dram_tensor("wmaint", [128, 36, 128], F16, kind="ExternalInput")
    baseY = nc.dram_tensor("baseY", [128, K2, NBLK], F32, kind="ExternalInput")
    baseX = nc.dram_tensor("baseX", [128, K2, NBLK], F32, kind="ExternalInput")
    bofft = nc.dram_tensor("bofft", [32, 1], F32, kind="ExternalInput")
    out = nc.dram_tensor("out", [2, 128, HS, W], F32, kind="ExternalOutput")

    with tile.TileContext(nc) as tc, ExitStack() as ctx:
        const = ctx.enter_context(tc.tile_pool(name="const", bufs=1))
        work = ctx.enter_context(tc.tile_pool(name="work", bufs=1))
        coeff = ctx.enter_context(tc.tile_pool(name="coeff", bufs=1))
        tmp = ctx.enter_context(tc.tile_pool(name="tmp", bufs=4))
        gpool = ctx.enter_context(tc.tile_pool(name="gpool", bufs=3))
        bpool = ctx.enter_context(tc.tile_pool(name="bpool", bufs=2))
        rhsp = ctx.enter_context(tc.tile_pool(name="rhsp", bufs=3))
        outp = ctx.enter_context(tc.tile_pool(name="outp", bufs=3))
        psB = ctx.enter_context(tc.tile_pool(name="psB", bufs=2, space="PSUM"))
        psC = ctx.enter_context(tc.tile_pool(name="psC", bufs=2, space="PSUM"))
        psO = ctx.enter_context(tc.tile_pool(name="psO", bufs=1, space="PSUM"))

        # ---- constants -----------------------------------------------------
        nc.gpsimd.load_library(library_config.mlp)
        ident16 = const.tile([128, 128], F16)
        make_identity(nc, ident16[:])
        identf = const.tile([128, 128], F32)
        make_identity(nc, identf[:])

        wom_sb = const.tile([128, 18, 32], F32R)
        nc.sync.dma_start(wom_sb[:], womt[:].rearrange("t c o -> c t o"))
        xs = []
        for ch in range(2):
            t = work.tile([128, 34, 130], F32R, tag=f"xs{ch}")
            nc.sync.dma_start(t[:, 0:11], xslab[ch][:, 0:11])
            xs.append(t)
        bY = const.tile([128, K2, NBLK], F32)
        nc.sync.dma_start(bY[:], baseY[:])
        bX = const.tile([128, K2, NBLK], F32)
        nc.sync.dma_start(bX[:], baseX[:])
        bo = const.tile([32, 1], F32)
        nc.sync.dma_start(bo[:], bofft[:])
        for ch in range(2):
            nc.sync.dma_start(xs[ch][:, 11:34], xslab[ch][:, 11:34])
        wm_sb = const.tile([128, 36, 128], F16)
        nc.sync.dma_start(wm_sb[:], wmaint[:])

        # broadcast-constant columns: [MAGIC, -MAGIC, 0, -1, 127, 126, 130]
        cst = const.tile([128, 8], F32)
        for i, v in enumerate((MAGIC, -MAGIC, 0.0, -1.0, float(H - 1),
                               float(H - 2), float(QD + 1))):
            nc.vector.memset(cst[:, i:i + 1], v)

        def cb(i):
            return cst[:, i:i + 1, None].broadcast_to([128, K2, 8])

        # ---- per-quarter pipeline with one-quarter lookahead ----------------
        def emit_prep(q):
            # 1. offset/mask conv for this quarter (8 rows, 2 N-blocks)
            sb_om = work.tile([32, 8 * W], F32, tag="sb_om", name="sb_om",
                              bufs=3)
            for lnb in range(2):
                nb = q * 2 + lnb
                ps = psC.tile([32, 512], F32, tag="omstage", name="ps_om")
                for t in range(18):
                    k, ch = divmod(t, 2)
                    ky, kx = divmod(k, 3)
                    rhs = xs[ch][:, nb * 4 + ky:nb * 4 + ky + 4, kx:kx + 128]
                    nc.tensor.matmul(
                        ps[:],
                        wom_sb[:, t, :],
                        rhs,
                        start=(t == 0),
                        stop=(t == 17),
                    )
                nc.scalar.activation(sb_om[:, lnb * 512:(lnb + 1) * 512],
                                     ps[:], AF.Identity, bias=bo[:])

            # 2a. transpose offsets to [pix, (blk, ch27)]
            t_off = coeff.tile([128, 27, 8], F32, tag="t_off", name="t_off",
                               bufs=3)
            for g in range(2):
                tp = psC.tile([128, 128], F32, tag="omstage", name="tp_o")
                for j in range(4):
                    bl = g * 4 + j
                    nc.tensor.transpose(
                        tp[:, j * 27:(j + 1) * 27],
                        sb_om[0:27, bl * 128:(bl + 1) * 128],
                        identf[0:27, 0:27],
                    )
                nc.scalar.copy(t_off[:, :, g * 4:(g + 1) * 4]
                               .rearrange('p c b -> p b c'), tp[:, 0:108])

            # 2b. coefficient + index pipeline (wide [128, 9, 8] ops)
            dy = t_off[:, 0:9, :]
            dx = t_off[:, 9:18, :]
            ml = t_off[:, 18:27, :]
            bYq = bY[:, :, q * 8:(q + 1) * 8]
            bXq = bX[:, :, q * 8:(q + 1) * 8]

            def ctile(tag):
                return coeff.tile([128, K2, 8], F32, tag=tag, name=tag,
                                  bufs=3)

            m = ctile('m')
            nc.scalar.activation(m[:], ml, AF.Sigmoid)

            pyp = ctile('pyp')
            nc.vector.tensor_add(pyp[:], dy, bYq)
            y0 = ctile('y0')
            nc.vector.tensor_tensor(y0[:], pyp[:], cb(0), OP.add)
            nc.vector.tensor_tensor(y0[:], y0[:], cb(1), OP.add)
            wy = ctile('wy')
            nc.vector.scalar_tensor_tensor(wy[:], pyp[:], 0.5, y0[:], OP.add,
                                           OP.subtract)
            pxp = ctile('pxp')
            nc.vector.tensor_add(pxp[:], dx, bXq)
            x0 = ctile('x0')
            nc.vector.tensor_tensor(x0[:], pxp[:], cb(0), OP.add)
            nc.vector.tensor_tensor(x0[:], x0[:], cb(1), OP.add)
            wx = ctile('wx')
            nc.vector.scalar_tensor_tensor(wx[:], pxp[:], 0.5, x0[:], OP.add,
                                           OP.subtract)

            def valid(srcv, lo_i, hi_i, tag):
                u = tmp.tile([128, K2, 8], F32, tag="vu", name="vu")
                nc.vector.tensor_tensor(u[:], srcv[:], cb(lo_i), OP.is_ge)
                v = tmp.tile([128, K2, 8], F32, tag="vv", name="vv")
                nc.vector.tensor_tensor(v[:], srcv[:], cb(hi_i), OP.is_le)
                w_ = ctile(tag)
                nc.vector.tensor_mul(w_[:], u[:], v[:])
                return w_

            vy0 = valid(y0, 2, 4, 'vy0')
            vy1 = valid(y0, 3, 5, 'vy1')
            vx0 = valid(x0, 2, 4, 'vx0')
            vx1 = valid(x0, 3, 5, 'vx1')

            y0c = ctile('y0c')
            nc.vector.tensor_tensor(y0c[:], y0[:], cb(3), OP.max)
            nc.vector.tensor_tensor(y0c[:], y0c[:], cb(4), OP.min)
            x0c = ctile('x0c')
            nc.vector.tensor_tensor(x0c[:], x0[:], cb(3), OP.max)
            nc.vector.tensor_tensor(x0c[:], x0c[:], cb(4), OP.min)

            def lerp_w(wv, v0, v1, tag):
                t_ = tmp.tile([128, K2, 8], F32, tag="lw", name="lw")
                nc.vector.tensor_mul(t_[:], wv[:], v0[:])
                f0 = ctile(tag + '0')
                nc.vector.tensor_sub(f0[:], v0[:], t_[:])
                f1 = ctile(tag + '1')
                nc.vector.tensor_mul(f1[:], wv[:], v1[:])
                return f0, f1

            fy0, fy1 = lerp_w(wy, vy0, vy1, 'fy')
            fx0, fx1 = lerp_w(wx, vx0, vx1, 'fx')

            g0 = ctile('g0')
            nc.vector.tensor_mul(g0[:], fy0[:], m[:])
            g1 = ctile('g1')
            nc.vector.tensor_mul(g1[:], fy1[:], m[:])
            a00 = ctile('a00')
            nc.vector.tensor_mul(a00[:], g0[:], fx0[:])
            a01 = ctile('a01')
            nc.vector.tensor_mul(a01[:], g0[:], fx1[:])
            a10 = ctile('a10')
            nc.vector.tensor_mul(a10[:], g1[:], fx0[:])
            a11 = ctile('a11')
            nc.vector.tensor_mul(a11[:], g1[:], fx1[:])

            idxf = ctile('idxf')
            nc.vector.scalar_tensor_tensor(idxf[:], y0c[:], float(QD), x0c[:],
                                           OP.mult, OP.add)
            nc.vector.tensor_tensor(idxf[:], idxf[:], cb(6), OP.add)
            idx16 = coeff.tile([128, K2, 8], I16, tag="idx16", name="idx16",
                               bufs=3)
            nc.vector.tensor_tensor(idx16[:], idxf[:], cb(2), OP.add)

            # 2c. fold idx to gather layout [16, (k, blk, g)] + replicate.
            # The gather ucode (queue 0) reads idxs from partitions 0-31
            # only (cores 0-1), so replicate just that far.
            idxg = coeff.tile([128, K2, 8, 8], I16, tag="idxg", name="idxg",
                              bufs=3)
            for g in range(8):
                srcv = idx16[g * 16:(g + 1) * 16, :, :]
                nc.sync.dma_start(idxg[0:16, :, :, g], srcv)
            nc.sync.dma_start(idxg[16:32], idxg[0:16])

            # fp16 copies of the blend coefficients (broadcast-TT operands)
            af = []
            for j, a in enumerate((a00, a01, a10)):
                t = coeff.tile([128, K2, 8], F16, tag=f"af{j}",
                               name=f"af{j}", bufs=3)
                nc.scalar.copy(t[:], a[:])
                af.append(t)

            return (a00, a01, a10, a11), af, idxg

        def emit_main(q, coefs):
            afull, af, idxg = coefs
            # 3-5. gather / blend / transpose+sum (PE) / GEMM
            po = [psO.tile([128, 512], F32, tag=f"po{i}", name=f"po{i}")
                  for i in range(4)]
            for k in range(K2):
                gbuf = gpool.tile([128, BPQ, 1024], F16, tag="gbuf")
                nc.gpsimd.dma_gather(
                    gbuf[:],
                    quad[:],
                    idxg[:, k, :, :],
                    num_idxs=BPQ * 128,
                    num_idxs_reg=BPQ * 128,
                    elem_size=1024,
                    single_packet=False,
                )
                # scaled corners tj[j] = gbuf[:, :, j] * a_j.  Corner 3 runs
                # as narrow per-block scale-copies on ACT; corners 0-2 as
                # wide single-port broadcast-TT muls on DVE (the 2-port TS
                # path would block on the SWDGE shared-port lock during
                # gathers).  The 4-corner sum happens on the PE: identity
                # matmuls accumulate the transposed corners in PSUM.
                tj = [bpool.tile([128, BPQ, 256], F16, tag=f"tj{j}",
                                 name=f"tj{j}", bufs=2) for j in range(4)]
                for bl in range(BPQ):
                    nc.scalar.activation(tj[3][:, bl, :],
                                         gbuf[:, bl, 768:1024], AF.Copy,
                                         scale=afull[3][:, k, bl:bl + 1])
                for j in range(3):
                    nc.vector.tensor_tensor(
                        tj[j][:], gbuf[:, :, j * 256:(j + 1) * 256],
                        af[j][:, k, :, None].broadcast_to([128, BPQ, 256]),
                        OP.mult)

                for j2 in range(2):
                    for ct in range(2):
                        tp = psB.tile([128, 512], F32, tag="stage",
                                      name="tp_b")
                        for r in range(4):
                            bl = j2 * 4 + r
                            for j in range(4):
                                nc.tensor.matmul(
                                    tp[:, r * 128:(r + 1) * 128],
                                    tj[j][:, bl, ct * 128:(ct + 1) * 128],
                                    ident16[:],
                                    start=(j == 0),
                                    stop=(j == 3),
                                )
                        rhs16 = rhsp.tile([128, 512], F16, tag="rhs",
                                          name="rhs")
                        nc.scalar.copy(rhs16[:], tp[:])
                        for ot in range(2):
                            widx = (k * 2 + ct) * 2 + ot
                            nc.tensor.matmul(
                                po[j2 * 2 + ot][:],
                                wm_sb[:, widx, :],
                                rhs16[:],
                                start=(k == 0 and ct == 0),
                                stop=(k == 8 and ct == 1),
                            )
            for j2 in range(2):
                og = q * 2 + j2
                for ot in range(2):
                    o_sb = outp.tile([128, 4, 128], F32, tag="osb")
                    nc.scalar.copy(o_sb[:], po[j2 * 2 + ot][:])
                    nc.sync.dma_start(out[ot, :, og * 4:(og + 1) * 4, :], o_sb[:])

        pending = []
        for q in range(NQRT):
            pending.append((q, emit_prep(q)))
            if len(pending) > 2 or q == 1:
                pq, pc = pending.pop(0)
                emit_main(pq, pc)
        while pending:
            pq, pc = pending.pop(0)
            emit_main(pq, pc)

    nc.finalize()
    return nc


# ----------------------------------------------------------------------------
# host-side data prep
# ----------------------------------------------------------------------------

def build_in_maps(x, w_conv, b_conv, w_off, b_off, w_mask, b_mask):
    x = np.ascontiguousarray(x, np.float32)

    # quad image per batch: quad[yq*129+xq, (j,c)] fp16
    quads = []
    for b in range(B):
        xp = np.zeros((H + 2, W + 2, CIN), np.float32)
        xp[1:-1, 1:-1] = x[b].transpose(1, 2, 0)
        q = np.empty((QD, QD, 4, CIN), np.float16)
        q[:, :, 0] = xp[0:QD, 0:QD]
        q[:, :, 1] = xp[0:QD, 1:QD + 1]
        q[:, :, 2] = xp[1:QD + 1, 0:QD]
        q[:, :, 3] = xp[1:QD + 1, 1:QD + 1]
        quads.append(np.ascontiguousarray(q.reshape(NQ, 1024)))

    # offset/mask weights, output channels reordered to [dy*9, dx*9, ml*9]
    wom = np.concatenate([w_off, w_mask], 0).reshape(27, CIN, K2)  # [o,c,k]
    perm = np.concatenate([np.arange(0, 18, 2), np.arange(1, 18, 2),
                           np.arange(18, 27)])
    womp = wom[perm]                                   # [27(dy,dx,ml), c, k]
    womt = np.zeros((18, 128, 32), np.float32)
    for t in range(18):
        k, ch = divmod(t, 2)
        womt[t, :, 0:27] = womp[:, ch * 128:(ch + 1) * 128, k].T
    bom = np.concatenate([b_off, b_mask]).astype(np.float32)[perm]
    bofft = np.zeros((32, 1), np.float32)
    bofft[0:27, 0] = bom

    # main weights [c, (k,ct,ot), o] fp16
    wc = w_conv.reshape(COUT, CIN, K2)
    wmaint = np.zeros((128, 36, 128), np.float16)
    for k in range(K2):
        for ct in range(2):
            for ot in range(2):
                widx = (k * 2 + ct) * 2 + ot
                wmaint[:, widx, :] = (
                    wc[ot * 128:(ot + 1) * 128, ct * 128:(ct + 1) * 128, k].T
                )

    ky = (np.arange(K2) // 3).astype(np.float32)
    kx = (np.arange(K2) % 3).astype(np.float32)
    bXc = np.zeros((128, K2, NBLK), np.float32)
    bXc[:] = (np.arange(128, dtype=np.float32)[:, None, None]
              + kx[None, :, None] - 1.5)

    in_maps = []
    for core in range(NCORE):
        b, slab = divmod(core, 4)
        h0 = slab * HS
        xsl = np.zeros((2, 128, 34, 130), np.float32)
        r_lo = max(0, h0 - 1)
        r_hi = min(H, h0 + HS + 1)
        xsl[:, :, (r_lo - (h0 - 1)):(r_hi - (h0 - 1)), 1:129] = (
            x[b].reshape(2, 128, H, W)[:, :, r_lo:r_hi, :]
        )
        bYc = np.zeros((128, K2, NBLK), np.float32)
        bYc[:] = ((h0 + np.arange(NBLK, dtype=np.float32))[None, None, :]
                  + ky[None, :, None] - 1.5)
        in_maps.append({
            "xslab": xsl,
            "quad": quads[b],
            "womt": womt,
            "wmaint": wmaint,
            "baseY": bYc,
            "baseX": np.ascontiguousarray(bXc),
            "bofft": bofft,
        })
    return in_maps


_PROGRAM = None
LAST_EXEC_NS = None
LAST_RESULTS = None


def kernel(x, w_conv, b_conv, w_off, b_off, w_mask, b_mask):
    global _PROGRAM, LAST_EXEC_NS, LAST_RESULTS
    in_maps = build_in_maps(x, w_conv, b_conv, w_off, b_off, w_mask, b_mask)
    if _PROGRAM is None:
        _PROGRAM = build_program()
    nc = _PROGRAM
    trace = bool(os.environ.get("DCN_TRACE"))
    res = run_bass_kernel_spmd(nc, in_maps, core_ids=list(range(NCORE)),
                               trace=trace)
    LAST_EXEC_NS = res.exec_time_ns
    LAST_RESULTS = res
    out = np.zeros((B, COUT, H, W), np.float32)
    for core in range(NCORE):
        b, slab = divmod(core, 4)
        h0 = slab * HS
        oc = res.results[core]["out"]  # [2, 128, 32, 128]
        out[b, 0:128, h0:h0 + HS, :] = oc[0]
        out[b, 128:256, h0:h0 + HS, :] = oc[1]
    # b_conv is zeros in the reference setup, but add anyway for correctness
    out += np.asarray(b_conv, np.float32)[None, :, None, None]
    return out



# revision 12
# speedup vs baseline: 1.3320x; 1.0911x over previous
"""Deformable Conv (DCNv2) Trainium2 Bass kernel.

Sharding: 8 cores = 2 batches x 4 H-slabs of 32 output rows each.

Per-core pipeline (single SPMD program, per-core data):
  1. offset/mask 3x3 conv as an 18-step fp32r GEMM on the PE from a
     CHW x-slab resident in SBUF.
  2. PE-transpose offsets to [pixel-partition, (row, k)] layout; compute
     bilinear blend coefficients (fp32) and gather indices (int16) with
     wide DVE/ACT ops. Validity of out-of-image corners is folded into the
     coefficients; addressing uses clamped indices, so arbitrary offsets
     are handled exactly.
  3. dma_gather (SWDGE) pulls, per (kernel-pos k, pixel), one 2KB row of a
     host-built "quad" image (4 bilinear corners x 256 channels, fp16) from
     HBM into [pixel, 4*256] SBUF tiles.
  4. DVE tensor_scalar/scalar_tensor_tensor chain blends the 4 corners with
     per-pixel (per-partition) fp32 coefficients (mask folded in) -> fp16.
  5. PE transposes blended tiles to [channel, pixel] and the main GEMM
     accumulates out[o,p] = sum_{c,k} W[o,c,k] * blended[c,k,p] in PSUM
     (fp16 x fp16 -> fp32).
"""
import os
import numpy as np
from contextlib import ExitStack

import concourse.bass as bass
import concourse.tile as tile
from concourse import bacc, mybir
from concourse.bass_utils import run_bass_kernel_spmd
from concourse.masks import make_identity
from concourse import library_config

F32 = mybir.dt.float32
F32R = mybir.dt.float32r
F16 = mybir.dt.float16
I16 = mybir.dt.int16
AF = mybir.ActivationFunctionType
OP = mybir.AluOpType

CIN = 256
COUT = 256
K2 = 9
H = W = 128
B = 2
HS = 32           # output rows per core
NCORE = 8
QD = H + 1        # quad grid dim (129)
NQ = QD * QD      # 16641 quad rows
MAGIC = 12582912.0  # 1.5 * 2**23 fp32 round-to-int magic

NBLK = 32         # pixel blocks (rows) per core
NQRT = 4          # quarters (8 rows each) per core
BPQ = 8           # blocks per quarter
# blend blocks 0..ACT_BLEND-1 use ACT for the 4 scalar muls (engine balance)
ACT_BLEND = 3


# ----------------------------------------------------------------------------
# device program
# ----------------------------------------------------------------------------

def build_program():
    nc = bacc.Bacc("TRN2", target_bir_lowering=False, debug=False)

    xslab = nc.dram_tensor("xslab", [2, 128, 34, 130], F32R, kind="ExternalInput")
    quad = nc.dram_tensor("quad", [NQ, 1024], F16, kind="ExternalInput")
    womt = nc.dram_tensor("womt", [18, 128, 32], F32R, kind="ExternalInput")
    wmaint = nc.dram_tensor("wmaint", [128, 36, 128], F16, kind="ExternalInput")
    baseY = nc.dram_tensor("baseY", [128, K2, NBLK], F32, kind="ExternalInput")
    baseX = nc.dram_tensor("baseX", [128, K2, NBLK], F32, kind="ExternalInput")
    bofft = nc.dram_tensor("bofft", [32, 1], F32, kind="ExternalInput")
    out = nc.dram_tensor("out", [2, 128, HS, W], F32, kind="ExternalOutput")

    with tile.TileContext(nc) as tc, ExitStack() as ctx:
        const = ctx.enter_context(tc.tile_pool(name="const", bufs=1))
        work = ctx.enter_context(tc.tile_pool(name="work", bufs=1))
        coeff = ctx.enter_context(tc.tile_pool(name="coeff", bufs=1))
        tmp = ctx.enter_context(tc.tile_pool(name="tmp", bufs=4))
        gpool = ctx.enter_context(tc.tile_pool(name="gpool", bufs=3))
        bpool = ctx.enter_context(tc.tile_pool(name="bpool", bufs=2))
        rhsp = ctx.enter_context(tc.tile_pool(name="rhsp", bufs=3))
        outp = ctx.enter_context(tc.tile_pool(name="outp", bufs=3))
        psB = ctx.enter_context(tc.tile_pool(name="psB", bufs=2, space="PSUM"))
        psC = ctx.enter_context(tc.tile_pool(name="psC", bufs=2, space="PSUM"))
        psO = ctx.enter_context(tc.tile_pool(name="psO", bufs=1, space="PSUM"))

        # ---- constants -----------------------------------------------------
        nc.gpsimd.load_library(library_config.mlp)
        ident16 = const.tile([128, 128], F16)
        make_identity(nc, ident16[:])
        identf = const.tile([128, 128], F32)
        make_identity(nc, identf[:])

        wom_sb = const.tile([128, 18, 32], F32R)
        nc.sync.dma_start(wom_sb[:], womt[:].rearrange("t c o -> c t o"))
        xs = []
        for ch in range(2):
            t = work.tile([128, 34, 130], F32R, tag=f"xs{ch}")
            nc.sync.dma_start(t[:, 0:11], xslab[ch][:, 0:11])
            xs.append(t)
        bY = const.tile([128, K2, NBLK], F32)
        nc.sync.dma_start(bY[:], baseY[:])
        bX = const.tile([128, K2, NBLK], F32)
        nc.sync.dma_start(bX[:], baseX[:])
        bo = const.tile([32, 1], F32)
        nc.sync.dma_start(bo[:], bofft[:])
        for ch in range(2):
            nc.sync.dma_start(xs[ch][:, 11:34], xslab[ch][:, 11:34])
        wm_sb = const.tile([128, 36, 128], F16)
        nc.sync.dma_start(wm_sb[:], wmaint[:])

        # broadcast-constant columns: [MAGIC, -MAGIC, 0, -1, 127, 126, 130]
        cst = const.tile([128, 8], F32)
        for i, v in enumerate((MAGIC, -MAGIC, 0.0, -1.0, float(H - 1),
                               float(H - 2), float(QD + 1))):
            nc.vector.memset(cst[:, i:i + 1], v)

        def cb(i):
            return cst[:, i:i + 1, None].broadcast_to([128, K2, 8])

        zi16 = const.tile([16, 1], I16)
        nc.vector.memset(zi16[:], 0)

        # ---- per-quarter pipeline with one-quarter lookahead ----------------
        def emit_prep(q):
            # 1. offset/mask conv for this quarter (8 rows, 2 N-blocks)
            sb_om = work.tile([32, 8 * W], F32, tag="sb_om", name="sb_om",
                              bufs=3)
            for lnb in range(2):
                nb = q * 2 + lnb
                ps = psC.tile([32, 512], F32, tag="omstage", name="ps_om")
                for t in range(18):
                    k, ch = divmod(t, 2)
                    ky, kx = divmod(k, 3)
                    rhs = xs[ch][:, nb * 4 + ky:nb * 4 + ky + 4, kx:kx + 128]
                    nc.tensor.matmul(
                        ps[:],
                        wom_sb[:, t, :],
                        rhs,
                        start=(t == 0),
                        stop=(t == 17),
                    )
                nc.scalar.activation(sb_om[:, lnb * 512:(lnb + 1) * 512],
                                     ps[:], AF.Identity, bias=bo[:])

            # 2a. transpose offsets to [pix, (blk, ch27)]
            t_off = coeff.tile([128, 27, 8], F32, tag="t_off", name="t_off",
                               bufs=3)
            for g in range(2):
                tp = psC.tile([128, 128], F32, tag="omstage", name="tp_o")
                for j in range(4):
                    bl = g * 4 + j
                    nc.tensor.transpose(
                        tp[:, j * 27:(j + 1) * 27],
                        sb_om[0:27, bl * 128:(bl + 1) * 128],
                        identf[0:27, 0:27],
                    )
                nc.scalar.copy(t_off[:, :, g * 4:(g + 1) * 4]
                               .rearrange('p c b -> p b c'), tp[:, 0:108])

            # 2b. coefficient + index pipeline (wide [128, 9, 8] ops)
            dy = t_off[:, 0:9, :]
            dx = t_off[:, 9:18, :]
            ml = t_off[:, 18:27, :]
            bYq = bY[:, :, q * 8:(q + 1) * 8]
            bXq = bX[:, :, q * 8:(q + 1) * 8]

            def ctile(tag):
                return coeff.tile([128, K2, 8], F32, tag=tag, name=tag,
                                  bufs=3)

            m = ctile('m')
            nc.scalar.activation(m[:], ml, AF.Sigmoid)

            pyp = ctile('pyp')
            nc.vector.tensor_add(pyp[:], dy, bYq)
            y0 = ctile('y0')
            nc.vector.tensor_tensor(y0[:], pyp[:], cb(0), OP.add)
            nc.vector.tensor_tensor(y0[:], y0[:], cb(1), OP.add)
            wy = ctile('wy')
            nc.vector.scalar_tensor_tensor(wy[:], pyp[:], 0.5, y0[:], OP.add,
                                           OP.subtract)
            pxp = ctile('pxp')
            nc.vector.tensor_add(pxp[:], dx, bXq)
            x0 = ctile('x0')
            nc.vector.tensor_tensor(x0[:], pxp[:], cb(0), OP.add)
            nc.vector.tensor_tensor(x0[:], x0[:], cb(1), OP.add)
            wx = ctile('wx')
            nc.vector.scalar_tensor_tensor(wx[:], pxp[:], 0.5, x0[:], OP.add,
                                           OP.subtract)

            def valid(srcv, lo_i, hi_i, tag):
                u = tmp.tile([128, K2, 8], F32, tag="vu", name="vu")
                nc.vector.tensor_tensor(u[:], srcv[:], cb(lo_i), OP.is_ge)
                v = tmp.tile([128, K2, 8], F32, tag="vv", name="vv")
                nc.vector.tensor_tensor(v[:], srcv[:], cb(hi_i), OP.is_le)
                w_ = ctile(tag)
                nc.vector.tensor_mul(w_[:], u[:], v[:])
                return w_

            vy0 = valid(y0, 2, 4, 'vy0')
            vy1 = valid(y0, 3, 5, 'vy1')
            vx0 = valid(x0, 2, 4, 'vx0')
            vx1 = valid(x0, 3, 5, 'vx1')

            y0c = ctile('y0c')
            nc.vector.tensor_tensor(y0c[:], y0[:], cb(3), OP.max)
            nc.vector.tensor_tensor(y0c[:], y0c[:], cb(4), OP.min)
            x0c = ctile('x0c')
            nc.vector.tensor_tensor(x0c[:], x0[:], cb(3), OP.max)
            nc.vector.tensor_tensor(x0c[:], x0c[:], cb(4), OP.min)

            def lerp_w(wv, v0, v1, tag):
                t_ = tmp.tile([128, K2, 8], F32, tag="lw", name="lw")
                nc.vector.tensor_mul(t_[:], wv[:], v0[:])
                f0 = ctile(tag + '0')
                nc.vector.tensor_sub(f0[:], v0[:], t_[:])
                f1 = ctile(tag + '1')
                nc.vector.tensor_mul(f1[:], wv[:], v1[:])
                return f0, f1

            fy0, fy1 = lerp_w(wy, vy0, vy1, 'fy')
            fx0, fx1 = lerp_w(wx, vx0, vx1, 'fx')

            g0 = ctile('g0')
            nc.vector.tensor_mul(g0[:], fy0[:], m[:])
            g1 = ctile('g1')
            nc.vector.tensor_mul(g1[:], fy1[:], m[:])
            a00 = ctile('a00')
            nc.vector.tensor_mul(a00[:], g0[:], fx0[:])
            a01 = ctile('a01')
            nc.vector.tensor_mul(a01[:], g0[:], fx1[:])
            a10 = ctile('a10')
            nc.vector.tensor_mul(a10[:], g1[:], fx0[:])
            a11 = ctile('a11')
            nc.vector.tensor_mul(a11[:], g1[:], fx1[:])

            idxf = ctile('idxf')
            nc.vector.scalar_tensor_tensor(idxf[:], y0c[:], float(QD), x0c[:],
                                           OP.mult, OP.add)
            nc.vector.tensor_tensor(idxf[:], idxf[:], cb(6), OP.add)
            idx16 = coeff.tile([128, K2, 8], I16, tag="idx16", name="idx16",
                               bufs=3)
            nc.vector.tensor_tensor(idx16[:], idxf[:], cb(2), OP.add)

            # 2c. fold idx to gather layout [16, (k, blk, g)] + replicate.
            # Two hops: 8 fully-contiguous partition-fold DMAs into
            # [16, g, k, blk], then one lock-free DVE bypass-copy to
            # transpose the free dims to [16, k, blk, g].  The gather
            # ucode (queue 0) reads idxs from partitions 0-31 only, so
            # replicate just that far.
            idxt = coeff.tile([16, 8, K2, 8], I16, tag="idxt", name="idxt",
                              bufs=3)
            for g in range(8):
                srcv = idx16[g * 16:(g + 1) * 16, :, :]
                nc.sync.dma_start(idxt[:, g], srcv)
            idxg = coeff.tile([128, K2, 8, 8], I16, tag="idxg", name="idxg",
                              bufs=3)
            nc.vector.tensor_tensor(
                idxg[0:16], idxt[:].rearrange('q g k b -> q k b g'),
                zi16[:, :, None, None].broadcast_to([16, K2, 8, 8]),
                OP.add)
            nc.sync.dma_start(idxg[16:32], idxg[0:16])

            # fp16 copies of the blend coefficients (broadcast-TT operands)
            af = []
            for j, a in enumerate((a00, a01, a10)):
                t = coeff.tile([128, K2, 8], F16, tag=f"af{j}",
                               name=f"af{j}", bufs=3)
                nc.scalar.copy(t[:], a[:])
                af.append(t)

            return (a00, a01, a10, a11), af, idxg

        def emit_main(q, coefs):
            afull, af, idxg = coefs
            # 3-5. gather / blend / transpose+sum (PE) / GEMM
            po = [psO.tile([128, 512], F32, tag=f"po{i}", name=f"po{i}")
                  for i in range(4)]
            for k in range(K2):
                gbuf = gpool.tile([128, BPQ, 1024], F16, tag="gbuf")
                nc.gpsimd.dma_gather(
                    gbuf[:],
                    quad[:],
                    idxg[:, k, :, :],
                    num_idxs=BPQ * 128,
                    num_idxs_reg=BPQ * 128,
                    elem_size=1024,
                    single_packet=False,
                )
                # scaled corners tj[j] = gbuf[:, :, j] * a_j.  Corner 3 runs
                # as narrow per-block scale-copies on ACT; corners 0-2 as
                # wide single-port broadcast-TT muls on DVE (the 2-port TS
                # path would block on the SWDGE shared-port lock during
                # gathers).  The 4-corner sum happens on the PE: identity
                # matmuls accumulate the transposed corners in PSUM.
                tj = [bpool.tile([128, BPQ, 256], F16, tag=f"tj{j}",
                                 name=f"tj{j}", bufs=2) for j in range(4)]
                for bl in range(BPQ):
                    nc.scalar.activation(tj[3][:, bl, :],
                                         gbuf[:, bl, 768:1024], AF.Copy,
                                         scale=afull[3][:, k, bl:bl + 1])
                for j in range(3):
                    nc.vector.tensor_tensor(
                        tj[j][:], gbuf[:, :, j * 256:(j + 1) * 256],
                        af[j][:, k, :, None].broadcast_to([128, BPQ, 256]),
                        OP.mult)

                for j2 in range(2):
                    for ct in range(2):
                        tp = psB.tile([128, 512], F32, tag="stage",
                                      name="tp_b")
                        for r in range(4):
                            bl = j2 * 4 + r
                            for j in range(4):
                                nc.tensor.matmul(
                                    tp[:, r * 128:(r + 1) * 128],
                                    tj[j][:, bl, ct * 128:(ct + 1) * 128],
                                    ident16[:],
                                    start=(j == 0),
                                    stop=(j == 3),
                                )
                        rhs16 = rhsp.tile([128, 512], F16, tag="rhs",
                                          name="rhs")
                        nc.scalar.copy(rhs16[:], tp[:])
                        for ot in range(2):
                            widx = (k * 2 + ct) * 2 + ot
                            nc.tensor.matmul(
                                po[j2 * 2 + ot][:],
                                wm_sb[:, widx, :],
                                rhs16[:],
                                start=(k == 0 and ct == 0),
                                stop=(k == 8 and ct == 1),
                            )
            for j2 in range(2):
                og = q * 2 + j2
                for ot in range(2):
                    o_sb = outp.tile([128, 4, 128], F32, tag="osb")
                    nc.scalar.copy(o_sb[:], po[j2 * 2 + ot][:])
                    nc.sync.dma_start(out[ot, :, og * 4:(og + 1) * 4, :], o_sb[:])

        pending = []
        for q in range(NQRT):
            pending.append((q, emit_prep(q)))
            if len(pending) > 2 or q == 1:
                pq, pc = pending.pop(0)
                emit_main(pq, pc)
        while pending:
            pq, pc = pending.pop(0)
            emit_main(pq, pc)

    nc.finalize()
    return nc


# ----------------------------------------------------------------------------
# host-side data prep
# ----------------------------------------------------------------------------

def build_in_maps(x, w_conv, b_conv, w_off, b_off, w_mask, b_mask):
    x = np.ascontiguousarray(x, np.float32)

    # quad image per batch: quad[yq*129+xq, (j,c)] fp16
    quads = []
    for b in range(B):
        xp = np.zeros((H + 2, W + 2, CIN), np.float32)
        xp[1:-1, 1:-1] = x[b].transpose(1, 2, 0)
        q = np.empty((QD, QD, 4, CIN), np.float16)
        q[:, :, 0] = xp[0:QD, 0:QD]
        q[:, :, 1] = xp[0:QD, 1:QD + 1]
        q[:, :, 2] = xp[1:QD + 1, 0:QD]
        q[:, :, 3] = xp[1:QD + 1, 1:QD + 1]
        quads.append(np.ascontiguousarray(q.reshape(NQ, 1024)))

    # offset/mask weights, output channels reordered to [dy*9, dx*9, ml*9]
    wom = np.concatenate([w_off, w_mask], 0).reshape(27, CIN, K2)  # [o,c,k]
    perm = np.concatenate([np.arange(0, 18, 2), np.arange(1, 18, 2),
                           np.arange(18, 27)])
    womp = wom[perm]                                   # [27(dy,dx,ml), c, k]
    womt = np.zeros((18, 128, 32), np.float32)
    for t in range(18):
        k, ch = divmod(t, 2)
        womt[t, :, 0:27] = womp[:, ch * 128:(ch + 1) * 128, k].T
    bom = np.concatenate([b_off, b_mask]).astype(np.float32)[perm]
    bofft = np.zeros((32, 1), np.float32)
    bofft[0:27, 0] = bom

    # main weights [c, (k,ct,ot), o] fp16
    wc = w_conv.reshape(COUT, CIN, K2)
    wmaint = np.zeros((128, 36, 128), np.float16)
    for k in range(K2):
        for ct in range(2):
            for ot in range(2):
                widx = (k * 2 + ct) * 2 + ot
                wmaint[:, widx, :] = (
                    wc[ot * 128:(ot + 1) * 128, ct * 128:(ct + 1) * 128, k].T
                )

    ky = (np.arange(K2) // 3).astype(np.float32)
    kx = (np.arange(K2) % 3).astype(np.float32)
    bXc = np.zeros((128, K2, NBLK), np.float32)
    bXc[:] = (np.arange(128, dtype=np.float32)[:, None, None]
              + kx[None, :, None] - 1.5)

    in_maps = []
    for core in range(NCORE):
        b, slab = divmod(core, 4)
        h0 = slab * HS
        xsl = np.zeros((2, 128, 34, 130), np.float32)
        r_lo = max(0, h0 - 1)
        r_hi = min(H, h0 + HS + 1)
        xsl[:, :, (r_lo - (h0 - 1)):(r_hi - (h0 - 1)), 1:129] = (
            x[b].reshape(2, 128, H, W)[:, :, r_lo:r_hi, :]
        )
        bYc = np.zeros((128, K2, NBLK), np.float32)
        bYc[:] = ((h0 + np.arange(NBLK, dtype=np.float32))[None, None, :]
                  + ky[None, :, None] - 1.5)
        in_maps.append({
            "xslab": xsl,
            "quad": quads[b],
            "womt": womt,
            "wmaint": wmaint,
            "baseY": bYc,
            "baseX": np.ascontiguousarray(bXc),
            "bofft": bofft,
        })
    return in_maps


_PROGRAM = None
LAST_EXEC_NS = None
LAST_RESULTS = None


def kernel(x, w_conv, b_conv, w_off, b_off, w_mask, b_mask):
    global _PROGRAM, LAST_EXEC_NS, LAST_RESULTS
    in_maps = build_in_maps(x, w_conv, b_conv, w_off, b_off, w_mask, b_mask)
    if _PROGRAM is None:
        _PROGRAM = build_program()
    nc = _PROGRAM
    trace = bool(os.environ.get("DCN_TRACE"))
    res = run_bass_kernel_spmd(nc, in_maps, core_ids=list(range(NCORE)),
                               trace=trace)
    LAST_EXEC_NS = res.exec_time_ns
    LAST_RESULTS = res
    out = np.zeros((B, COUT, H, W), np.float32)
    for core in range(NCORE):
        b, slab = divmod(core, 4)
        h0 = slab * HS
        oc = res.results[core]["out"]  # [2, 128, 32, 128]
        out[b, 0:128, h0:h0 + HS, :] = oc[0]
        out[b, 128:256, h0:h0 + HS, :] = oc[1]
    # b_conv is zeros in the reference setup, but add anyway for correctness
    out += np.asarray(b_conv, np.float32)[None, :, None, None]
    return out



# revision 13
# speedup vs baseline: 1.4093x; 1.0580x over previous
"""Deformable Conv (DCNv2) Trainium2 Bass kernel.

Sharding: 8 cores = 2 batches x 4 H-slabs of 32 output rows each.

Per-core pipeline (single SPMD program, per-core data):
  1. offset/mask 3x3 conv as an 18-step fp32r GEMM on the PE from a
     CHW x-slab resident in SBUF.
  2. PE-transpose offsets to [pixel-partition, (row, k)] layout; compute
     bilinear blend coefficients (fp32) and gather indices (int16) with
     wide DVE/ACT ops. Validity of out-of-image corners is folded into the
     coefficients; addressing uses clamped indices, so arbitrary offsets
     are handled exactly.
  3. dma_gather (SWDGE) pulls, per (kernel-pos k, pixel), one 2KB row of a
     host-built "quad" image (4 bilinear corners x 256 channels, fp16) from
     HBM into [pixel, 4*256] SBUF tiles.
  4. DVE tensor_scalar/scalar_tensor_tensor chain blends the 4 corners with
     per-pixel (per-partition) fp32 coefficients (mask folded in) -> fp16.
  5. PE transposes blended tiles to [channel, pixel] and the main GEMM
     accumulates out[o,p] = sum_{c,k} W[o,c,k] * blended[c,k,p] in PSUM
     (fp16 x fp16 -> fp32).
"""
import os
import numpy as np
from contextlib import ExitStack

import concourse.bass as bass
import concourse.tile as tile
from concourse import bacc, mybir
from concourse.bass_utils import run_bass_kernel_spmd
from concourse.masks import make_identity
from concourse import library_config

F32 = mybir.dt.float32
F32R = mybir.dt.float32r
F16 = mybir.dt.float16
I16 = mybir.dt.int16
AF = mybir.ActivationFunctionType
OP = mybir.AluOpType

CIN = 256
COUT = 256
K2 = 9
H = W = 128
B = 2
HS = 32           # output rows per core
NCORE = 8
QD = H + 1        # quad grid dim (129)
NQ = QD * QD      # 16641 quad rows
MAGIC = 12582912.0  # 1.5 * 2**23 fp32 round-to-int magic

NBLK = 32         # pixel blocks (rows) per core
NQRT = 4          # quarters (8 rows each) per core
BPQ = 8           # blocks per quarter
# blend blocks 0..ACT_BLEND-1 use ACT for the 4 scalar muls (engine balance)
ACT_BLEND = 3


# ----------------------------------------------------------------------------
# device program
# ----------------------------------------------------------------------------

def build_program():
    nc = bacc.Bacc("TRN2", target_bir_lowering=False, debug=False,
                   num_swdge_queues=2)

    xslab = nc.dram_tensor("xslab", [2, 128, 34, 130], F32R, kind="ExternalInput")
    quad = nc.dram_tensor("quad", [NQ, 1024], F16, kind="ExternalInput")
    womt = nc.dram_tensor("womt", [18, 128, 32], F32R, kind="ExternalInput")
    wmaint = nc.dram_tensor("wmaint", [128, 36, 128], F16, kind="ExternalInput")
    baseY = nc.dram_tensor("baseY", [128, K2, NBLK], F32, kind="ExternalInput")
    baseX = nc.dram_tensor("baseX", [128, K2, NBLK], F32, kind="ExternalInput")
    bofft = nc.dram_tensor("bofft", [32, 1], F32, kind="ExternalInput")
    out = nc.dram_tensor("out", [2, 128, HS, W], F32, kind="ExternalOutput")

    with tile.TileContext(nc) as tc, ExitStack() as ctx:
        const = ctx.enter_context(tc.tile_pool(name="const", bufs=1))
        work = ctx.enter_context(tc.tile_pool(name="work", bufs=1))
        coeff = ctx.enter_context(tc.tile_pool(name="coeff", bufs=1))
        tmp = ctx.enter_context(tc.tile_pool(name="tmp", bufs=4))
        gpool = ctx.enter_context(tc.tile_pool(name="gpool", bufs=3))
        bpool = ctx.enter_context(tc.tile_pool(name="bpool", bufs=2))
        rhsp = ctx.enter_context(tc.tile_pool(name="rhsp", bufs=3))
        outp = ctx.enter_context(tc.tile_pool(name="outp", bufs=3))
        psB = ctx.enter_context(tc.tile_pool(name="psB", bufs=2, space="PSUM"))
        psC = ctx.enter_context(tc.tile_pool(name="psC", bufs=2, space="PSUM"))
        psO = ctx.enter_context(tc.tile_pool(name="psO", bufs=1, space="PSUM"))

        # ---- constants -----------------------------------------------------
        nc.gpsimd.load_library(library_config.mlp)
        ident16 = const.tile([128, 128], F16)
        make_identity(nc, ident16[:])
        identf = const.tile([128, 128], F32)
        make_identity(nc, identf[:])

        wom_sb = const.tile([128, 18, 32], F32R)
        nc.sync.dma_start(wom_sb[:], womt[:].rearrange("t c o -> c t o"))
        xs = []
        for ch in range(2):
            t = work.tile([128, 34, 130], F32R, tag=f"xs{ch}")
            nc.sync.dma_start(t[:, 0:11], xslab[ch][:, 0:11])
            xs.append(t)
        bY = const.tile([128, K2, NBLK], F32)
        nc.sync.dma_start(bY[:], baseY[:])
        bX = const.tile([128, K2, NBLK], F32)
        nc.sync.dma_start(bX[:], baseX[:])
        bo = const.tile([32, 1], F32)
        nc.sync.dma_start(bo[:], bofft[:])
        for ch in range(2):
            nc.sync.dma_start(xs[ch][:, 11:34], xslab[ch][:, 11:34])
        wm_sb = const.tile([128, 36, 128], F16)
        nc.sync.dma_start(wm_sb[:], wmaint[:])

        # broadcast-constant columns: [MAGIC, -MAGIC, 0, -1, 127, 126, 130]
        cst = const.tile([128, 8], F32)
        for i, v in enumerate((MAGIC, -MAGIC, 0.0, -1.0, float(H - 1),
                               float(H - 2), float(QD + 1))):
            nc.vector.memset(cst[:, i:i + 1], v)

        def cb(i):
            return cst[:, i:i + 1, None].broadcast_to([128, K2, 8])

        zi16 = const.tile([16, 1], I16)
        nc.vector.memset(zi16[:], 0)

        # ---- per-quarter pipeline with one-quarter lookahead ----------------
        def emit_prep(q):
            # 1. offset/mask conv for this quarter (8 rows, 2 N-blocks)
            sb_om = work.tile([32, 8 * W], F32, tag="sb_om", name="sb_om",
                              bufs=3)
            for lnb in range(2):
                nb = q * 2 + lnb
                ps = psC.tile([32, 512], F32, tag="omstage", name="ps_om")
                for t in range(18):
                    k, ch = divmod(t, 2)
                    ky, kx = divmod(k, 3)
                    rhs = xs[ch][:, nb * 4 + ky:nb * 4 + ky + 4, kx:kx + 128]
                    nc.tensor.matmul(
                        ps[:],
                        wom_sb[:, t, :],
                        rhs,
                        start=(t == 0),
                        stop=(t == 17),
                    )
                nc.scalar.activation(sb_om[:, lnb * 512:(lnb + 1) * 512],
                                     ps[:], AF.Identity, bias=bo[:])

            # 2a. transpose offsets to [pix, (blk, ch27)]
            t_off = coeff.tile([128, 27, 8], F32, tag="t_off", name="t_off",
                               bufs=3)
            for g in range(2):
                tp = psC.tile([128, 128], F32, tag="omstage", name="tp_o")
                for j in range(4):
                    bl = g * 4 + j
                    nc.tensor.transpose(
                        tp[:, j * 27:(j + 1) * 27],
                        sb_om[0:27, bl * 128:(bl + 1) * 128],
                        identf[0:27, 0:27],
                    )
                nc.scalar.copy(t_off[:, :, g * 4:(g + 1) * 4]
                               .rearrange('p c b -> p b c'), tp[:, 0:108])

            # 2b. coefficient + index pipeline (wide [128, 9, 8] ops)
            dy = t_off[:, 0:9, :]
            dx = t_off[:, 9:18, :]
            ml = t_off[:, 18:27, :]
            bYq = bY[:, :, q * 8:(q + 1) * 8]
            bXq = bX[:, :, q * 8:(q + 1) * 8]

            def ctile(tag):
                return coeff.tile([128, K2, 8], F32, tag=tag, name=tag,
                                  bufs=3)

            m = ctile('m')
            nc.scalar.activation(m[:], ml, AF.Sigmoid)

            pyp = ctile('pyp')
            nc.vector.tensor_add(pyp[:], dy, bYq)
            y0 = ctile('y0')
            nc.vector.tensor_tensor(y0[:], pyp[:], cb(0), OP.add)
            nc.vector.tensor_tensor(y0[:], y0[:], cb(1), OP.add)
            wy = ctile('wy')
            nc.vector.scalar_tensor_tensor(wy[:], pyp[:], 0.5, y0[:], OP.add,
                                           OP.subtract)
            pxp = ctile('pxp')
            nc.vector.tensor_add(pxp[:], dx, bXq)
            x0 = ctile('x0')
            nc.vector.tensor_tensor(x0[:], pxp[:], cb(0), OP.add)
            nc.vector.tensor_tensor(x0[:], x0[:], cb(1), OP.add)
            wx = ctile('wx')
            nc.vector.scalar_tensor_tensor(wx[:], pxp[:], 0.5, x0[:], OP.add,
                                           OP.subtract)

            def valid(srcv, lo_i, hi_i, tag):
                u = tmp.tile([128, K2, 8], F32, tag="vu", name="vu")
                nc.vector.tensor_tensor(u[:], srcv[:], cb(lo_i), OP.is_ge)
                v = tmp.tile([128, K2, 8], F32, tag="vv", name="vv")
                nc.vector.tensor_tensor(v[:], srcv[:], cb(hi_i), OP.is_le)
                w_ = ctile(tag)
                nc.vector.tensor_mul(w_[:], u[:], v[:])
                return w_

            vy0 = valid(y0, 2, 4, 'vy0')
            vy1 = valid(y0, 3, 5, 'vy1')
            vx0 = valid(x0, 2, 4, 'vx0')
            vx1 = valid(x0, 3, 5, 'vx1')

            y0c = ctile('y0c')
            nc.vector.tensor_tensor(y0c[:], y0[:], cb(3), OP.max)
            nc.vector.tensor_tensor(y0c[:], y0c[:], cb(4), OP.min)
            x0c = ctile('x0c')
            nc.vector.tensor_tensor(x0c[:], x0[:], cb(3), OP.max)
            nc.vector.tensor_tensor(x0c[:], x0c[:], cb(4), OP.min)

            def lerp_w(wv, v0, v1, tag):
                t_ = tmp.tile([128, K2, 8], F32, tag="lw", name="lw")
                nc.vector.tensor_mul(t_[:], wv[:], v0[:])
                f0 = ctile(tag + '0')
                nc.vector.tensor_sub(f0[:], v0[:], t_[:])
                f1 = ctile(tag + '1')
                nc.vector.tensor_mul(f1[:], wv[:], v1[:])
                return f0, f1

            fy0, fy1 = lerp_w(wy, vy0, vy1, 'fy')
            fx0, fx1 = lerp_w(wx, vx0, vx1, 'fx')

            g0 = ctile('g0')
            nc.vector.tensor_mul(g0[:], fy0[:], m[:])
            g1 = ctile('g1')
            nc.vector.tensor_mul(g1[:], fy1[:], m[:])
            a00 = ctile('a00')
            nc.vector.tensor_mul(a00[:], g0[:], fx0[:])
            a01 = ctile('a01')
            nc.vector.tensor_mul(a01[:], g0[:], fx1[:])
            a10 = ctile('a10')
            nc.vector.tensor_mul(a10[:], g1[:], fx0[:])
            a11 = ctile('a11')
            nc.vector.tensor_mul(a11[:], g1[:], fx1[:])

            idxf = ctile('idxf')
            nc.vector.scalar_tensor_tensor(idxf[:], y0c[:], float(QD), x0c[:],
                                           OP.mult, OP.add)
            nc.vector.tensor_tensor(idxf[:], idxf[:], cb(6), OP.add)
            idx16 = coeff.tile([128, K2, 8], I16, tag="idx16", name="idx16",
                               bufs=3)
            nc.vector.tensor_tensor(idx16[:], idxf[:], cb(2), OP.add)

            # 2c. fold idx to gather layout [16, (k, blk, g)] + replicate.
            # Two hops: 8 fully-contiguous partition-fold DMAs into
            # [16, g, k, blk], then one lock-free DVE bypass-copy to
            # transpose the free dims to [16, k, blk, g].  The gather
            # ucode (queue 0) reads idxs from partitions 0-31 only, so
            # replicate just that far.
            idxt = coeff.tile([16, 8, K2, 8], I16, tag="idxt", name="idxt",
                              bufs=3)
            for g in range(8):
                srcv = idx16[g * 16:(g + 1) * 16, :, :]
                nc.sync.dma_start(idxt[:, g], srcv)
            idxg = coeff.tile([128, K2, 8, 8], I16, tag="idxg", name="idxg",
                              bufs=3)
            nc.vector.tensor_tensor(
                idxg[0:16], idxt[:].rearrange('q g k b -> q k b g'),
                zi16[:, :, None, None].broadcast_to([16, K2, 8, 8]),
                OP.add)
            nc.sync.dma_start(idxg[16:32], idxg[0:16])
            nc.sync.dma_start(idxg[32:64], idxg[0:32])

            # fp16 copies of the blend coefficients (broadcast-TT operands)
            af = []
            for j, a in enumerate((a00, a01, a10)):
                t = coeff.tile([128, K2, 8], F16, tag=f"af{j}",
                               name=f"af{j}", bufs=3)
                nc.scalar.copy(t[:], a[:])
                af.append(t)

            return (a00, a01, a10, a11), af, idxg

        def emit_main(q, coefs):
            afull, af, idxg = coefs
            # 3-5. gather / blend / transpose+sum (PE) / GEMM
            po = [psO.tile([128, 512], F32, tag=f"po{i}", name=f"po{i}")
                  for i in range(4)]
            for k in range(K2):
                gbuf = gpool.tile([128, BPQ, 1024], F16, tag="gbuf")
                nc.gpsimd.dma_gather(
                    gbuf[:],
                    quad[:],
                    idxg[:, k, :, :],
                    num_idxs=BPQ * 128,
                    num_idxs_reg=BPQ * 128,
                    elem_size=1024,
                    single_packet=False,
                    queue_num=k % 2,
                )
                # scaled corners tj[j] = gbuf[:, :, j] * a_j.  Corner 3 runs
                # as narrow per-block scale-copies on ACT; corners 0-2 as
                # wide single-port broadcast-TT muls on DVE (the 2-port TS
                # path would block on the SWDGE shared-port lock during
                # gathers).  The 4-corner sum happens on the PE: identity
                # matmuls accumulate the transposed corners in PSUM.
                tj = [bpool.tile([128, BPQ, 256], F16, tag=f"tj{j}",
                                 name=f"tj{j}", bufs=2) for j in range(4)]
                for bl in range(BPQ):
                    nc.scalar.activation(tj[3][:, bl, :],
                                         gbuf[:, bl, 768:1024], AF.Copy,
                                         scale=afull[3][:, k, bl:bl + 1])
                for j in range(3):
                    nc.vector.tensor_tensor(
                        tj[j][:], gbuf[:, :, j * 256:(j + 1) * 256],
                        af[j][:, k, :, None].broadcast_to([128, BPQ, 256]),
                        OP.mult)

                for j2 in range(2):
                    for ct in range(2):
                        tp = psB.tile([128, 512], F32, tag="stage",
                                      name="tp_b")
                        for r in range(4):
                            bl = j2 * 4 + r
                            for j in range(4):
                                nc.tensor.matmul(
                                    tp[:, r * 128:(r + 1) * 128],
                                    tj[j][:, bl, ct * 128:(ct + 1) * 128],
                                    ident16[:],
                                    start=(j == 0),
                                    stop=(j == 3),
                                )
                        rhs16 = rhsp.tile([128, 512], F16, tag="rhs",
                                          name="rhs")
                        nc.scalar.copy(rhs16[:], tp[:])
                        for ot in range(2):
                            widx = (k * 2 + ct) * 2 + ot
                            nc.tensor.matmul(
                                po[j2 * 2 + ot][:],
                                wm_sb[:, widx, :],
                                rhs16[:],
                                start=(k == 0 and ct == 0),
                                stop=(k == 8 and ct == 1),
                            )
            for j2 in range(2):
                og = q * 2 + j2
                for ot in range(2):
                    o_sb = outp.tile([128, 4, 128], F32, tag="osb")
                    nc.scalar.copy(o_sb[:], po[j2 * 2 + ot][:])
                    nc.sync.dma_start(out[ot, :, og * 4:(og + 1) * 4, :], o_sb[:])

        pending = []
        for q in range(NQRT):
            pending.append((q, emit_prep(q)))
            if len(pending) > 2 or q == 1:
                pq, pc = pending.pop(0)
                emit_main(pq, pc)
        while pending:
            pq, pc = pending.pop(0)
            emit_main(pq, pc)

    nc.finalize()
    return nc


# ----------------------------------------------------------------------------
# host-side data prep
# ----------------------------------------------------------------------------

def build_in_maps(x, w_conv, b_conv, w_off, b_off, w_mask, b_mask):
    x = np.ascontiguousarray(x, np.float32)

    # quad image per batch: quad[yq*129+xq, (j,c)] fp16
    quads = []
    for b in range(B):
        xp = np.zeros((H + 2, W + 2, CIN), np.float32)
        xp[1:-1, 1:-1] = x[b].transpose(1, 2, 0)
        q = np.empty((QD, QD, 4, CIN), np.float16)
        q[:, :, 0] = xp[0:QD, 0:QD]
        q[:, :, 1] = xp[0:QD, 1:QD + 1]
        q[:, :, 2] = xp[1:QD + 1, 0:QD]
        q[:, :, 3] = xp[1:QD + 1, 1:QD + 1]
        quads.append(np.ascontiguousarray(q.reshape(NQ, 1024)))

    # offset/mask weights, output channels reordered to [dy*9, dx*9, ml*9]
    wom = np.concatenate([w_off, w_mask], 0).reshape(27, CIN, K2)  # [o,c,k]
    perm = np.concatenate([np.arange(0, 18, 2), np.arange(1, 18, 2),
                           np.arange(18, 27)])
    womp = wom[perm]                                   # [27(dy,dx,ml), c, k]
    womt = np.zeros((18, 128, 32), np.float32)
    for t in range(18):
        k, ch = divmod(t, 2)
        womt[t, :, 0:27] = womp[:, ch * 128:(ch + 1) * 128, k].T
    bom = np.concatenate([b_off, b_mask]).astype(np.float32)[perm]
    bofft = np.zeros((32, 1), np.float32)
    bofft[0:27, 0] = bom

    # main weights [c, (k,ct,ot), o] fp16
    wc = w_conv.reshape(COUT, CIN, K2)
    wmaint = np.zeros((128, 36, 128), np.float16)
    for k in range(K2):
        for ct in range(2):
            for ot in range(2):
                widx = (k * 2 + ct) * 2 + ot
                wmaint[:, widx, :] = (
                    wc[ot * 128:(ot + 1) * 128, ct * 128:(ct + 1) * 128, k].T
                )

    ky = (np.arange(K2) // 3).astype(np.float32)
    kx = (np.arange(K2) % 3).astype(np.float32)
    bXc = np.zeros((128, K2, NBLK), np.float32)
    bXc[:] = (np.arange(128, dtype=np.float32)[:, None, None]
              + kx[None, :, None] - 1.5)

    in_maps = []
    for core in range(NCORE):
        b, slab = divmod(core, 4)
        h0 = slab * HS
        xsl = np.zeros((2, 128, 34, 130), np.float32)
        r_lo = max(0, h0 - 1)
        r_hi = min(H, h0 + HS + 1)
        xsl[:, :, (r_lo - (h0 - 1)):(r_hi - (h0 - 1)), 1:129] = (
            x[b].reshape(2, 128, H, W)[:, :, r_lo:r_hi, :]
        )
        bYc = np.zeros((128, K2, NBLK), np.float32)
        bYc[:] = ((h0 + np.arange(NBLK, dtype=np.float32))[None, None, :]
                  + ky[None, :, None] - 1.5)
        in_maps.append({
            "xslab": xsl,
            "quad": quads[b],
            "womt": womt,
            "wmaint": wmaint,
            "baseY": bYc,
            "baseX": np.ascontiguousarray(bXc),
            "bofft": bofft,
        })
    return in_maps


_PROGRAM = None
LAST_EXEC_NS = None
LAST_RESULTS = None


def kernel(x, w_conv, b_conv, w_off, b_off, w_mask, b_mask):
    global _PROGRAM, LAST_EXEC_NS, LAST_RESULTS
    in_maps = build_in_maps(x, w_conv, b_conv, w_off, b_off, w_mask, b_mask)
    if _PROGRAM is None:
        _PROGRAM = build_program()
    nc = _PROGRAM
    trace = bool(os.environ.get("DCN_TRACE"))
    res = run_bass_kernel_spmd(nc, in_maps, core_ids=list(range(NCORE)),
                               trace=trace)
    LAST_EXEC_NS = res.exec_time_ns
    LAST_RESULTS = res
    out = np.zeros((B, COUT, H, W), np.float32)
    for core in range(NCORE):
        b, slab = divmod(core, 4)
        h0 = slab * HS
        oc = res.results[core]["out"]  # [2, 128, 32, 128]
        out[b, 0:128, h0:h0 + HS, :] = oc[0]
        out[b, 128:256, h0:h0 + HS, :] = oc[1]
    # b_conv is zeros in the reference setup, but add anyway for correctness
    out += np.asarray(b_conv, np.float32)[None, :, None, None]
    return out

